# revision 1
# baseline (speedup 1.0000x reference)
"""Bass/Trainium2 kernel for DocRE bilinear segment-reduce model.

Shapes (hardcoded): B=4, L=1024, H=768, NH=12, E=24, M=4, P=552, NL=97, BLK=64.
Sharding: 8 cores = (batch b = core//2) x (half of the 552 head-tail pairs).
One SPMD program; all per-core differences flow through input data
(gathered rows + one-hot matrices built on host from the integer indices).
"""

import dataclasses
import numpy as np
import ml_dtypes

import concourse.bass as bass
import concourse.bacc as bacc
import concourse.tile as tile
from concourse import mybir
from concourse.bass_utils import run_bass_kernel_spmd

B, L, H, NH, E, M, P, NL, BLK = 4, 1024, 768, 12, 24, 4, 552, 97, 64
G = H // BLK            # 12 blocks
R = P // 2              # 276 rows per core
EM = E * M              # 96 gathered mentions
HL = NH * L             # 12288
K = H * BLK             # 49152 bilinear contraction
F32 = mybir.dt.float32
BF16 = mybir.dt.bfloat16

_CACHE = {}


def _bcast_src(ap, rep):
    """Source AP [1, F] -> [1, (rep, F)] with step-0 free dim (DMA broadcast)."""
    return dataclasses.replace(ap, ap=[ap.ap[0], [0, rep]] + ap.ap[1:])


def _build_program():
    nc = bacc.Bacc("TRN2", target_bir_lowering=False, debug=False, num_devices=8)
    dp = nc.declare_dram_parameter
    EMG = dp("EMG", [EM, H], F32, isOutput=False)        # gathered mention embeddings
    SUME = dp("SUME", [EM, 128], BF16, isOutput=False)     # mask one-hot  (logsumexp sum)
    AMG = dp("AMG", [EM, HL], BF16, isOutput=False)      # gathered mention attn rows (h-major)
    SUMW = dp("SUMW", [EM, 128], BF16, isOutput=False)     # mask/denom/sqrt(NH) one-hot
    OHH = dp("OHH", [128, R], BF16, isOutput=False)         # head-entity one-hot
    OHT = dp("OHT", [128, R], BF16, isOutput=False)         # tail-entity one-hot
    SEQ = dp("SEQ", [L, H], BF16, isOutput=False)         # sequence_output[b]
    WHT = dp("WHT", [2 * H, H], BF16, isOutput=False)     # Wh.T
    WTT = dp("WTT", [2 * H, H], BF16, isOutput=False)     # Wt.T
    # Wb.T packed partition-major: WBT_P[p, load, j, n] = wbtperm[load*1024+j*128+p, n]
    # so each per-partition DMA run is 8*97*2 = 1552 B (>=512 B avoids the 2x
    # small-descriptor penalty).
    WBT = dp("WBT", [128, 48, 8, NL], BF16, isOutput=False)
    BHS = dp("BHS", [128, 6], F32, isOutput=False)       # bh as [128,6] per o-chunk
    BTS = dp("BTS", [128, 6], F32, isOutput=False)
    BBS = dp("BBS", [NL, 1], F32, isOutput=False)
    # sel variants packed partition-major: SEL_P[p, w, v, c]
    SEL = dp("SEL", [128, 8, 8, 128], BF16, isOutput=False)
    OUT = dp("OUT", [NL, R], F32, isOutput=True)         # logits^T

    with tile.TileContext(nc) as tc:
        with (
            tc.tile_pool(name="persist", bufs=1) as pp,
            tc.tile_pool(name="stream", bufs=6) as sp,
            tc.tile_pool(name="wstream", bufs=5) as wp,
            tc.tile_pool(name="wbt", bufs=6) as wbp,
            tc.tile_pool(name="rep", bufs=8) as repp,
            tc.tile_pool(name="blt", bufs=16) as bltp,
        ):
            # ---- load persistent small inputs
            def load(name, ap, shape, tag, dt=F32):
                t = pp.tile(shape, dt, tag=tag)
                nc.sync.dma_start(t[:], ap)
                return t

            emg = load("EMG", EMG[:], [EM, H], "emg")
            sume = load("SUME", SUME[:], [EM, 128], "sume", BF16)
            sumw = load("SUMW", SUMW[:], [EM, 128], "sumw", BF16)
            ohh = load("OHH", OHH[:], [128, R], "ohh", BF16)
            oht = load("OHT", OHT[:], [128, R], "oht", BF16)
            bhs = load("BHS", BHS[:], [128, 6], "bhs")
            bts = load("BTS", BTS[:], [128, 6], "bts")
            bbs = load("BBS", BBS[:], [NL, 1], "bbs")
            selt = []
            for w in range(8):
                st8 = pp.tile([128, 8 * 128], BF16, tag=f"sel8_{w}", name=f"sel8_{w}")
                nc.gpsimd.dma_start(
                    st8[:].rearrange("p (v c) -> p v c", v=8), SEL[:, w, :, :])
                selt.extend(st8[:, v * 128:(v + 1) * 128] for v in range(8))
            seqt = [load("SEQ", SEQ[q * 128:(q + 1) * 128, :], [128, H], f"seq{q}", BF16)
                    for q in range(8)]
            ones = pp.tile([128, 128], F32, tag="ones", name="ones")
            nc.vector.memset(ones[:], 1.0)
            onesb = pp.tile([128, 1], BF16, tag="onesb", name="onesb")
            nc.vector.memset(onesb[:], 1.0)

            # ---- phase 1: entity embeddings = ln(sum_m mask * exp(m_emb))
            expt = pp.tile([EM, H], BF16, tag="expt", name="expt")
            nc.scalar.activation(expt[:], emg[:], mybir.ActivationFunctionType.Exp)
            eet = pp.tile([128, H], BF16, tag="eet", name="eet")
            with tc.tile_pool(name="ps1", bufs=2, space="PSUM") as ps1:
              for half in range(2):
                pe = ps1.tile([128, 384], F32, tag="ee_ps", name="ee_ps")
                nc.tensor.matmul(pe[:], sume[:], expt[:, half * 384:(half + 1) * 384],
                                 start=True, stop=True)
                nc.scalar.activation(eet[:, half * 384:(half + 1) * 384], pe[:],
                                     mybir.ActivationFunctionType.Ln)

            # ---- phase 2: entity attentions A2 [E, (h,l)] = sum_m (mask/denom/sqrt NH) * att
            a2t = pp.tile([128, HL], BF16, tag="a2t", name="a2t")
            with tc.tile_pool(name="ps2", bufs=4, space="PSUM") as ps2:
              for i2 in range(HL // 1024):
                amg_c = sp.tile([EM, 1024], BF16, tag="amg", name="amg")
                nc.scalar.dma_start(amg_c[:], AMG[:, i2 * 1024:(i2 + 1) * 1024])
                for half2 in range(2):
                    i = i2 * 2 + half2
                    pa = ps2.tile([128, 512], F32, tag="a2_ps", name="a2_ps")
                    nc.tensor.matmul(pa[:], sumw[:],
                                     amg_c[:, half2 * 512:(half2 + 1) * 512],
                                     start=True, stop=True)
                    if i % 2 == 0:
                        nc.vector.tensor_copy(a2t[:, i * 512:(i + 1) * 512], pa[:])
                    else:
                        nc.scalar.copy(a2t[:, i * 512:(i + 1) * 512], pa[:])

            # ---- phase 3: hs^T / ts^T gathers  [128d, R] x 6
            hst, tst = [], []
            with tc.tile_pool(name="ps3", bufs=4, space="PSUM") as ps3:
              for oc in range(6):
                for si, (oh, dst_list, tag) in enumerate(
                        ((ohh, hst, "hs"), (oht, tst, "ts"))):
                    rg = ((oc * 2 + si) % 4) * 32
                    pg = ps3.tile([128, R], F32, tag="gat_ps", name="gat_ps")
                    nc.tensor.matmul(pg[:],
                                     eet[rg:rg + E, oc * 128:(oc + 1) * 128],
                                     oh[rg:rg + E, :],
                                     start=True, stop=True,
                                     tile_position=(rg, 0))
                    t = pp.tile([128, R], BF16, tag=f"{tag}{oc}", name=f"{tag}{oc}")
                    nc.scalar.copy(t[:], pg[:])
                    dst_list.append(t)

            # ---- phase 4: ht_att (pre-normalization) per l-chunk
            htacc = []
            with tc.tile_pool(name="ps4", bufs=2, space="PSUM") as ps4:
              for q in range(8):
                acc = pp.tile([128, R], BF16, tag=f"ht{q}", name=f"ht{q}")
                w4 = sp.tile([128, NH, R], F32, tag="w4", name="w4", bufs=2)
                for hp in range(NH // 2):
                    # two heads share one 2-bank PSUM tile -> one mul, one evac.
                    # The 4 gathers run in 4 PE row-groups concurrently (K=24).
                    hh2 = ps4.tile([128, 1024], F32, tag="hh_ps", name="hh_ps", bufs=2)
                    tt2 = ps4.tile([128, 1024], F32, tag="tt_ps", name="tt_ps", bufs=2)
                    for kk in range(2):
                        h = hp * 2 + kk
                        lsl = slice(h * L + q * 128, h * L + (q + 1) * 128)
                        rg_h, rg_t = kk * 64, kk * 64 + 32
                        nc.tensor.matmul(hh2[:, kk * 512:kk * 512 + R],
                                         a2t[rg_h:rg_h + E, lsl],
                                         ohh[rg_h:rg_h + E, :],
                                         start=True, stop=True,
                                         tile_position=(rg_h, 0))
                        nc.tensor.matmul(tt2[:, kk * 512:kk * 512 + R],
                                         a2t[rg_t:rg_t + E, lsl],
                                         oht[rg_t:rg_t + E, :],
                                         start=True, stop=True,
                                         tile_position=(rg_t, 0))
                    # DVE can read at most one PSUM operand: evac T via ScalarE
                    tview = dataclasses.replace(
                        tt2[:], ap=[tt2[:].ap[0], [512, 2], [1, R]])
                    hview = dataclasses.replace(
                        hh2[:], ap=[hh2[:].ap[0], [512, 2], [1, R]])
                    tsb = sp.tile([128, 2, R], F32, tag="tsb", name="tsb")
                    nc.scalar.copy(tsb[:], tview)
                    nc.vector.tensor_tensor(w4[:, hp * 2:hp * 2 + 2, :], hview,
                                            tsb[:], mybir.AluOpType.mult)
                # tree-sum over the 12 heads; 2 adds on GpSimd (SBUF-only ok)
                nc.vector.tensor_add(w4[:, 0:6, :], w4[:, 0:6, :], w4[:, 6:12, :])
                nc.gpsimd.tensor_add(w4[:, 0:3, :], w4[:, 0:3, :], w4[:, 3:6, :])
                nc.vector.tensor_add(w4[:, 0, :], w4[:, 0, :], w4[:, 1, :])
                nc.gpsimd.tensor_add(acc[:], w4[:, 0, :], w4[:, 2, :])
                htacc.append(acc)

            # ---- phase 5: 1/(sum_l ht + 1e-5), broadcast to 128 partitions
            invd = pp.tile([128, R], F32, tag="invd", name="invd")
            with tc.tile_pool(name="ps5", bufs=1, space="PSUM") as ps5:
                psum_s = ps5.tile([1, R], F32, tag="s_ps", name="s_ps")
                for q in range(8):
                    nc.tensor.matmul(psum_s[:], onesb[:], htacc[q][:],
                                     start=(q == 0), stop=(q == 7))
                invd1 = pp.tile([1, R], F32, tag="invd1", name="invd1")
                nc.vector.tensor_scalar_add(invd1[:], psum_s[:], 1e-5)
                nc.vector.reciprocal(invd1[:], invd1[:])
                pb = ps5.tile([128, R], F32, tag="invd_ps", name="invd_ps")
                nc.tensor.matmul(pb[:], ones[0:1, :], invd1[:], start=True, stop=True)
                nc.scalar.copy(invd[:], pb[:])

            # ---- phase 6: rs^T chunks (normalization folded into evac)
            rst = []
            with tc.tile_pool(name="ps6", bufs=2, space="PSUM") as ps6:
              for dc in range(6):
                pr = ps6.tile([128, R], F32, tag="rs_ps", name="rs_ps")
                for q in range(8):
                    nc.tensor.matmul(pr[:], seqt[q][:, dc * 128:(dc + 1) * 128],
                                     htacc[q][:], start=(q == 0), stop=(q == 7))
                t = pp.tile([128, R], BF16, tag=f"rs{dc}", name=f"rs{dc}")
                nc.vector.tensor_mul(t[:], pr[:], invd[:])
                rst.append(t)

            # ---- phase 7: projections zh^T = tanh(Wh^T @ [hs; rs] + bh), same for zt
            zht, ztt = [], []
            for (wdram, inv, bias, out_list, tag) in (
                    (WHT, hst, bhs, zht, "zh"), (WTT, tst, bts, ztt, "zt")):
              with tc.tile_pool(name=f"ps7{tag}", bufs=1, space="PSUM") as ps7:
                pps = [ps7.tile([128, R], F32, tag=f"{tag}_ps{oc}", name=f"{tag}_ps{oc}") for oc in range(6)]
                for k2 in range(6):
                    wt2 = wp.tile([128, 2, H], BF16, tag="wproj", name="wproj")
                    nc.sync.dma_start(
                        wt2[:],
                        wdram[k2 * 256:(k2 + 1) * 256, :].rearrange("(j p) n -> p j n", p=128))
                    for kk in range(2):
                        k = k2 * 2 + kk
                        rhs = inv[k] if k < 6 else rst[k - 6]
                        for oc in range(6):
                            nc.tensor.matmul(pps[oc][:],
                                             wt2[:, kk, oc * 128:(oc + 1) * 128],
                                             rhs[:], start=(k == 0), stop=(k == 11))
                for oc in range(6):
                    t = pp.tile([128, R], BF16, tag=f"{tag}{oc}", name=f"{tag}{oc}")
                    nc.scalar.activation(t[:], pps[oc][:],
                                         mybir.ActivationFunctionType.Tanh,
                                         bias=bias[:, oc:oc + 1])
                    out_list.append(t)

            # ---- phase 8: zt replicated per group g: [zt_g; zt_g]
            ztr = []
            for g in range(G):
                src = ztt[g // 2][(g % 2) * 64:(g % 2) * 64 + 64, :]
                t = pp.tile([128, R], BF16, tag=f"ztr{g}", name=f"ztr{g}")
                nc.sync.dma_start(t[0:64, :], src)
                nc.sync.dma_start(t[64:128, :], src)
                ztr.append(t)

            # ---- phase 9: bilinear logits^T = sum_k WbT[k,:]^T * bl^T[k,:]
            ps9_cm = tc.tile_pool(name="ps9", bufs=1, space="PSUM")
            ps9 = ps9_cm.__enter__()
            ps9r_cm = tc.tile_pool(name="ps9r", bufs=7, space="PSUM")
            ps9r = ps9r_cm.__enter__()
            lt = ps9.tile([NL, R], F32, tag="lt_ps", name="lt_ps")
            nchunks = G * 32
            ci = 0
            for gp in range(G // 2):
                zsrc = zht[gp]
                for i0 in range(0, 64, 2):
                  for gg in range(2):
                    g = gp * 2 + gg
                    # K=64 rep matmul in row-group gg*64: even/odd-g chunks
                    # interleave so the two reps run concurrently on the PE
                    sv = selt[gg * 32 + i0 // 2]
                    half = slice(gg * 64, gg * 64 + 64)
                    rep = ps9r.tile([128, R], F32, tag="rep_ps", name="rep_ps")
                    nc.tensor.matmul(rep[:], sv[half, :], zsrc[half, :],
                                     start=True, stop=True,
                                     tile_position=(gg * 64, 0))
                    blt = bltp.tile([128, R], BF16, tag="blt", name="blt")
                    if ci % 3 == 2:
                        # keep DVE free: evac via ACT, multiply on GpSimd
                        rsb = repp.tile([128, R], BF16, tag="rsb", name="rsb")
                        nc.scalar.copy(rsb[:], rep[:])
                        nc.gpsimd.tensor_mul(blt[:], rsb[:], ztr[g][:])
                    else:
                        nc.vector.tensor_mul(blt[:], rep[:], ztr[g][:])
                    if ci % 8 == 0:
                        # WBT is host-permuted to this loop's chunk order
                        wbc8 = wbp.tile([128, 8, NL], BF16, tag="wbc", name="wbc")
                        qeng = nc.sync if (ci // 8) % 2 == 0 else nc.scalar
                        qeng.dma_start(wbc8[:], WBT[:, ci // 8, :, :])
                    nc.tensor.matmul(lt[:], wbc8[:, ci % 8, :], blt[:],
                                     start=(ci == 0), stop=(ci == nchunks - 1))
                    ci += 1

            lout = pp.tile([NL, R], F32, tag="lout", name="lout")
            nc.vector.tensor_scalar_add(lout[:], lt[:], bbs[:, 0:1])
            nc.sync.dma_start(OUT[:], lout[:])
            ps9r_cm.__exit__(None, None, None)
            ps9_cm.__exit__(None, None, None)

    nc.finalize()
    return nc


def _wbt_perm(Wb):
    wbt = Wb.T  # [K, NL]
    order = []
    for gp in range(G // 2):
        for i0 in range(0, 64, 2):
            for gg in range(2):
                base = (gp * 2 + gg) * 4096 + i0 * 64
                order.append(np.arange(base, base + 128))
    perm = np.concatenate(order)
    w = np.ascontiguousarray(wbt[perm]).astype(ml_dtypes.bfloat16)  # [K, NL]
    # pack partition-major: [p, load, j, n]
    return np.ascontiguousarray(
        w.reshape(48, 8, 128, NL).transpose(2, 0, 1, 3))


def _sel_variants():
    sel = np.zeros((64, 128, 128), np.float32)
    for v in range(64):
        half, i0 = v // 32, (v % 32) * 2
        for p in range(128):
            sel[v, half * 64 + i0 + p // 64, p] = 1.0
    s = sel.reshape(8, 8, 128, 128).astype(ml_dtypes.bfloat16)
    # pack partition-major: [p, w, v, c]
    return np.ascontiguousarray(s.transpose(2, 0, 1, 3))


def _prep_core_inputs(c, sequence_output, attention, mention_mask, Wh, bh, Wt, bt,
                      Wb, bb, mention_idx, hts):
    b, half = c // 2, c % 2
    seq_b = np.ascontiguousarray(sequence_output[b])              # [L, H]
    idx = mention_idx[b].astype(np.int64).reshape(EM)             # [96]
    mask = mention_mask[b].astype(np.float32)                     # [E, M]
    denom = mask.sum(-1)                                          # [E]

    emg = np.ascontiguousarray(seq_b[idx])                        # [96, H]
    amg = np.ascontiguousarray(
        attention[b][:, idx, :].transpose(1, 0, 2).reshape(EM, HL))

    sume = np.zeros((EM, 128), np.float32)
    sumw = np.zeros((EM, 128), np.float32)
    s = 1.0 / np.sqrt(np.float32(NH))
    for e in range(E):
        for m in range(M):
            for rg in range(4):
                sume[e * M + m, rg * 32 + e] = mask[e, m]
                sumw[e * M + m, rg * 32 + e] = mask[e, m] / denom[e] * s
    # unused gap partitions (rows 24-31 of each group): keep their exp-sums
    # positive so the Ln over the full [128, .] tile stays finite
    for rg in range(4):
        sume[0, rg * 32 + E:rg * 32 + 32] = 1.0

    hts_c = hts[b, half * R:(half + 1) * R].astype(np.int64)      # [R, 2]
    ohh = np.zeros((128, R), np.float32)
    oht = np.zeros((128, R), np.float32)
    for rg in range(4):
        ohh[rg * 32 + hts_c[:, 0], np.arange(R)] = 1.0
        oht[rg * 32 + hts_c[:, 1], np.arange(R)] = 1.0

    return {
        "EMG": emg, "SUME": sume.astype(ml_dtypes.bfloat16),
        "AMG": amg.astype(ml_dtypes.bfloat16),
        "SUMW": sumw.astype(ml_dtypes.bfloat16),
        "OHH": ohh.astype(ml_dtypes.bfloat16), "OHT": oht.astype(ml_dtypes.bfloat16), "SEQ": seq_b.astype(ml_dtypes.bfloat16),
        "WHT": np.ascontiguousarray(Wh.T).astype(ml_dtypes.bfloat16), "WTT": np.ascontiguousarray(Wt.T).astype(ml_dtypes.bfloat16),
        "WBT": _wbt_perm(Wb),
        "BHS": np.ascontiguousarray(bh.reshape(6, 128).T),
        "BTS": np.ascontiguousarray(bt.reshape(6, 128).T),
        "BBS": bb.reshape(NL, 1).astype(np.float32),
        "SEL": _sel_variants(),
    }


def kernel(sequence_output, attention, mention_mask, Wh, bh, Wt, bt, Wb, bb,
           mention_idx, hts):
    if "nc" not in _CACHE:
        _CACHE["nc"] = _build_program()
    nc = _CACHE["nc"]

    args = (np.asarray(sequence_output, np.float32), np.asarray(attention, np.float32),
            np.asarray(mention_mask, np.float32), np.asarray(Wh, np.float32),
            np.asarray(bh, np.float32), np.asarray(Wt, np.float32),
            np.asarray(bt, np.float32), np.asarray(Wb, np.float32),
            np.asarray(bb, np.float32), np.asarray(mention_idx),
            np.asarray(hts))
    in_maps = [_prep_core_inputs(c, *args) for c in range(8)]
    try:
        res = run_bass_kernel_spmd(nc, in_maps, list(range(8))).results
    except Exception:
        # transient NRT_EXEC_UNIT_UNRECOVERABLE has been observed on the
        # first execution of a freshly loaded NEFF; retry once
        res = run_bass_kernel_spmd(nc, in_maps, list(range(8))).results

    out = np.empty((B, P, NL), np.float32)
    for c in range(8):
        b, half = c // 2, c % 2
        out[b, half * R:(half + 1) * R, :] = np.asarray(res[c]["OUT"]).T
    return out



# revision 23
# speedup vs baseline: 1.4696x; 1.4696x over previous
"""Bass/Trainium2 kernel for DocRE bilinear segment-reduce model (v2).

Shapes (hardcoded): B=4, L=1024, H=768, NH=12, E=24, M=4, P=552, NL=97, BLK=64.
Sharding: 8 cores = (batch b = core//2) x (half of the 552 head-tail pairs).

Key structure vs v1:
- ph1: entity embeddings produced directly transposed (eetT [128d, 24e] x 6).
- ph2 folded into ph4: host combines mask weights with pair one-hots
  (WHH/WTT [96, R]) so entity-attention gathers contract K=96 directly.
- ph7 split: entity-side projection (24 cols) before the pair gather;
  rs-side projected with K=768.
- ph9: bilinear chunk layout (8 i's x 16 j's per 128-partition chunk).
  zh/zt replication tiles materialized in SBUF bf16 via DRAM round-trip
  DMAs; multiplies are all-bf16 DVE 2x_1p wide ops (4 chunks each) split
  with Pool; accumulate = 384 K=128 matmuls into one PSUM bank.
"""

import dataclasses
import numpy as np
import ml_dtypes

import concourse.bass as bass
import concourse.bacc as bacc
import concourse.tile as tile
from concourse import mybir
from concourse.bass_utils import run_bass_kernel_spmd

B, L, H, NH, E, M, P, NL, BLK = 4, 1024, 768, 12, 24, 4, 552, 97, 64
G = H // BLK            # 12 groups
R = P // 2              # 276 rows per core
EM = E * M              # 96 gathered mentions
HL = NH * L             # 12288
F32 = mybir.dt.float32
BF16 = mybir.dt.bfloat16
BF = ml_dtypes.bfloat16

# ph4 per-q engine mix for the 6 head-pair rounds:
#   'a' = t-evac only, DVE multiplies from PSUM fp32 (ACT 0.6us, DVE 1.0us)
#   'b' = full evac, DVE bf16 2x multiply     (ACT 1.2us, DVE 0.35us)
#   'c' = full evac, Pool bf16 multiply       (ACT 1.2us, Pool 1.2us)
PH4_MIX = ['a', 'a', 'a', 'a', 'c', 'c']
# ph9: units u with u % MOD == MOD-1 multiply on Pool, rest on DVE.
PH9_POOL_MOD = 5        # 19 of 96 units -> Pool


def _bcast(ap, n):
    """Insert a step-0 dim after the partition dim: [p, F] -> [p, n, F]."""
    return dataclasses.replace(ap, ap=[ap.ap[0], [0, n]] + ap.ap[1:])


def _build_program():
    nc = bacc.Bacc("TRN2", target_bir_lowering=False, debug=False, num_devices=8)
    dp = nc.declare_dram_parameter
    EMG = dp("EMG", [EM, H], F32, isOutput=False)       # gathered mention embeds
    SUME = dp("SUME", [EM, E], BF16, isOutput=False)    # mask one-hot
    AMG = dp("AMG", [EM, HL], BF16, isOutput=False)     # gathered attn rows (h-major)
    WHH = dp("WHH", [EM, R], BF16, isOutput=False)      # mask/denom/sqrtNH x head one-hot
    WTT = dp("WTT", [EM, R], BF16, isOutput=False)
    OHH = dp("OHH", [E, R], BF16, isOutput=False)       # head-entity one-hot
    OHT = dp("OHT", [E, R], BF16, isOutput=False)
    SEQ = dp("SEQ", [L, H], BF16, isOutput=False)
    W1H = dp("W1H", [H, H], BF16, isOutput=False)       # Wh[:, :768].T
    W2H = dp("W2H", [H, H], BF16, isOutput=False)       # Wh[:, 768:].T
    W1T = dp("W1T", [H, H], BF16, isOutput=False)       # Wt[perm, :768].T
    W2T = dp("W2T", [H, H], BF16, isOutput=False)
    WBT = dp("WBT", [128, 12, 32, NL], BF16, isOutput=False)  # permuted Wb.T
    BHS = dp("BHS", [128, 6], F32, isOutput=False)
    BTS = dp("BTS", [128, 6], F32, isOutput=False)
    BBS = dp("BBS", [NL, 1], F32, isOutput=False)
    XD = dp("XD", [8, G, 8, R], BF16, isOutput=False)   # scratch: zh reorder
    ZD = dp("ZD", [16, G, 4, R], BF16, isOutput=False)  # scratch: zt reorder
    OUT = dp("OUT", [NL, R], F32, isOutput=True)        # logits^T

    XDs, ZDs = 26496, 13248  # per-i / per-j row sizes (G*8*R, G*4*R)

    with tile.TileContext(nc) as tc:
        with (
            tc.tile_pool(name="persist", bufs=1) as pp,
            tc.tile_pool(name="wbt", bufs=8) as wbp,
            tc.tile_pool(name="blt", bufs=6) as bltp,
        ):
            # ---- persistent small loads (sync queue, program order = priority)
            def load(name_ap, shape, tag, dt=F32, eng=nc.sync):
                t = pp.tile(shape, dt, tag=tag, name=tag)
                eng.dma_start(t[:], name_ap)
                return t

            emg = load(EMG[:], [EM, H], "emg")
            sume = load(SUME[:], [EM, E], "sume", BF16)
            # amgp opened before w1p (LIFO pool closing: w1p closes first)
            amgp_cm = tc.tile_pool(name="amgp", bufs=1)
            amgp = amgp_cm.__enter__()
            w1p_cm = tc.tile_pool(name="w1p", bufs=1)
            w1p = w1p_cm.__enter__()
            whh = load(WHH[:], [EM, R], "whh", BF16)
            wtt = load(WTT[:], [EM, R], "wtt", BF16)
            ohh = load(OHH[:], [E, R], "ohh", BF16)
            oht = load(OHT[:], [E, R], "oht", BF16)
            bhs = load(BHS[:], [128, 6], "bhs")
            bts = load(BTS[:], [128, 6], "bts")
            bbs = load(BBS[:], [NL, 1], "bbs")
            # attention rows, 4 heads per load (pool scoped to ph4)
            amgq = []
            for hq in range(3):
                t = amgp.tile([EM, 4, L], BF16, tag=f"amg{hq}", name=f"amg{hq}")
                nc.sync.dma_start(
                    t[:], AMG[:, hq * 4 * L:(hq + 1) * 4 * L].rearrange(
                        "m (h l) -> m h l", h=4))
                amgq.append(t)
            amgt = [amgq[hp // 2][:, (hp % 2) * 2:(hp % 2) * 2 + 2, :]
                    for hp in range(6)]

            def load_pair_at(dram, off, pool, tag):
                t = pool.tile([128, 2, H], BF16, tag=tag, name=tag)
                base = dram[:]
                src = dataclasses.replace(
                    base, offset=base.offset + off * 128 * H,
                    ap=[[H, 128], [128 * H, 2], [1, H]])
                nc.sync.dma_start(t[:], src)
                return t

            # W1 weights next: ph7a is the first PE consumer after ph1
            w1tp = [load_pair_at(W1T, 2 * i, w1p, f"w1t{i}") for i in range(3)]
            w1hp = [load_pair_at(W1H, 2 * i, w1p, f"w1h{i}") for i in range(3)]
            w1tt = [w1tp[dk // 2][:, dk % 2, :] for dk in range(6)]
            w1ht = [w1hp[dk // 2][:, dk % 2, :] for dk in range(6)]
            seqp = [load_pair_at(SEQ, 2 * i, pp, f"seq{i}") for i in range(4)]
            seqt = [seqp[q // 2][:, q % 2, :] for q in range(8)]
            w2tp = [load_pair_at(W2T, 2 * i, pp, f"w2t{i}") for i in range(3)]
            w2hp = [load_pair_at(W2H, 2 * i, pp, f"w2h{i}") for i in range(3)]
            w2tt = [w2tp[dk // 2][:, dk % 2, :] for dk in range(6)]
            w2ht = [w2hp[dk // 2][:, dk % 2, :] for dk in range(6)]
            onesb = pp.tile([128, 1], BF16, tag="onesb", name="onesb")
            nc.vector.memset(onesb[:], 1.0)
            ones1 = pp.tile([1, 128], F32, tag="ones1", name="ones1")
            nc.vector.memset(ones1[:], 1.0)

            # 8 WBT loads prefetched on sync after all early-phase loads:
            # their transfers fill the DMA-idle ph4 window.
            wbts = []
            for i in range(8):
                t = wbp.tile([128, 32, NL], BF16, tag="wbc", name=f"wbc{i}")
                nc.sync.dma_start(t[:], WBT[:, i, :, :])
                wbts.append(t)

            # ---- ph1: eetT[d, e] = ln(sum_m sume[m, e] * exp(emg[m, d]))
            expt = pp.tile([EM, H], BF16, tag="expt", name="expt")
            nc.scalar.activation(expt[:], emg[:], mybir.ActivationFunctionType.Exp)
            eetT = []
            with tc.tile_pool(name="ps1", bufs=2, space="PSUM") as ps1:
                for dc in range(6):
                    pe = ps1.tile([128, E], F32, tag="ee_ps", name="ee_ps")
                    nc.tensor.matmul(pe[:], expt[:, dc * 128:(dc + 1) * 128],
                                     sume[:], start=True, stop=True)
                    t = pp.tile([128, E], BF16, tag=f"eetT{dc}", name=f"eetT{dc}")
                    nc.scalar.activation(t[:], pe[:], mybir.ActivationFunctionType.Ln)
                    eetT.append(t)

            # ---- ph4: ht_att accumulation per l-chunk q
            # hA/tA gathers K=96 (ph2 folded into WHH/WTT), 2 heads per round.
            htacc = []
            evp_cm = tc.tile_pool(name="evac", bufs=6)
            evp = evp_cm.__enter__()
            with (
                tc.tile_pool(name="ps4", bufs=2, space="PSUM") as ps4,
                tc.tile_pool(name="w4p", bufs=2) as w4p,
            ):
                for q in range(8):
                    w4 = w4p.tile([128, NH, R], BF16, tag="w4", name="w4")
                    for hp in range(6):
                        hh2 = ps4.tile([128, 2, 512], F32, tag="hh_ps", name="hh_ps")
                        tt2 = ps4.tile([128, 2, 512], F32, tag="tt_ps", name="tt_ps")
                        for kk in range(2):
                            amg_c = amgt[hp][:, kk, q * 128:(q + 1) * 128]
                            nc.tensor.matmul(hh2[:, kk, 0:R], amg_c, whh[:],
                                             start=True, stop=True)
                            nc.tensor.matmul(tt2[:, kk, 0:R], amg_c, wtt[:],
                                             start=True, stop=True)
                        # evacuate t-side (ACT), multiply per PH4_MIX
                        mode = PH4_MIX[hp]
                        tsb = evp.tile([128, 2, R], BF16, tag="tsb", name="tsb")
                        nc.scalar.copy(tsb[:], tt2[:, :, 0:R])
                        if mode == 'a':
                            nc.vector.tensor_tensor(
                                w4[:, hp * 2:hp * 2 + 2, :], hh2[:, :, 0:R],
                                tsb[:], mybir.AluOpType.mult)
                        else:
                            hsb = evp.tile([128, 2, R], BF16, tag="hsb", name="hsb")
                            nc.scalar.copy(hsb[:], hh2[:, :, 0:R])
                            eng = nc.vector if mode == 'b' else nc.gpsimd
                            eng.tensor_tensor(
                                w4[:, hp * 2:hp * 2 + 2, :], hsb[:], tsb[:],
                                mybir.AluOpType.mult)
                    # tree-sum over 12 heads (bf16, 2x): 6+3+(1+2)
                    acc = pp.tile([128, R], BF16, tag=f"ht{q}", name=f"ht{q}")
                    nc.vector.tensor_add(w4[:, 0:6, :], w4[:, 0:6, :], w4[:, 6:12, :])
                    nc.gpsimd.tensor_add(w4[:, 0:3, :], w4[:, 0:3, :], w4[:, 3:6, :])
                    nc.vector.tensor_add(w4[:, 0, :], w4[:, 0, :], w4[:, 1, :])
                    nc.vector.tensor_add(acc[:], w4[:, 0, :], w4[:, 2, :])
                    htacc.append(acc)
            evp_cm.__exit__(None, None, None)

            # ---- ph7a: entity-side projections ph_eT[e, o] (h and t)
            pheT, pteT = [], []
            for (w1l, outl, tag) in ((w1tt, pteT, "pte"), (w1ht, pheT, "phe")):
                with tc.tile_pool(name=f"ps7a{tag}", bufs=1, space="PSUM") as ps7a:
                    pgs = [ps7a.tile([E, 128], F32, tag=f"{tag}ps{oc}",
                                     name=f"{tag}ps{oc}") for oc in range(6)]
                    for dk in range(6):
                        for oc in range(6):
                            nc.tensor.matmul(pgs[oc][:], eetT[dk][:],
                                             w1l[dk][:, oc * 128:(oc + 1) * 128],
                                             start=(dk == 0), stop=(dk == 5))
                    for oc in range(6):
                        t = pp.tile([E, 128], BF16, tag=f"{tag}{oc}", name=f"{tag}{oc}")
                        nc.scalar.copy(t[:], pgs[oc][:])
                        outl.append(t)
            w1p_cm.__exit__(None, None, None)

            w1p_cm.__exit__(None, None, None)
            amgp_cm.__exit__(None, None, None)

            # ---- ph5: invd = 1/(sum_l ht + 1e-5), broadcast to 128 partitions
            invd = pp.tile([128, R], F32, tag="invd", name="invd")
            with tc.tile_pool(name="ps5", bufs=1, space="PSUM") as ps5:
                psum_s = ps5.tile([1, R], F32, tag="s_ps", name="s_ps")
                for q in range(8):
                    nc.tensor.matmul(psum_s[:], onesb[:], htacc[q][:],
                                     start=(q == 0), stop=(q == 7))
                invd1 = pp.tile([1, R], F32, tag="invd1", name="invd1")
                nc.vector.tensor_scalar_add(invd1[:], psum_s[:], 1e-5)
                nc.vector.reciprocal(invd1[:], invd1[:])
                pb = ps5.tile([128, R], F32, tag="invd_ps", name="invd_ps")
                nc.tensor.matmul(pb[:], ones1[:], invd1[:], start=True, stop=True)
                nc.scalar.copy(invd[:], pb[:])

            # ---- ph6: rs^T chunks (normalization folded into evac)
            rst = []
            with tc.tile_pool(name="ps6", bufs=2, space="PSUM") as ps6:
                for dc in range(6):
                    pr = ps6.tile([128, R], F32, tag="rs_ps", name="rs_ps")
                    for q in range(8):
                        nc.tensor.matmul(pr[:], seqt[q][:, dc * 128:(dc + 1) * 128],
                                         htacc[q][:], start=(q == 0), stop=(q == 7))
                    t = pp.tile([128, R], BF16, tag=f"rs{dc}", name=f"rs{dc}")
                    nc.vector.tensor_mul(t[:], pr[:], invd[:])
                    rst.append(t)

            # ---- ph7b + ph8: zT projections with per-oc stage1/stage2
            # interleaving. zh first; its rep-stage2 reads overlap the zt
            # projection compute; ztr-stage2-gp issues right after zt-oc=gp.
            # rep_all[p, g, bi, r] = zh[g*64 + bi*8 + p//16]
            # ztr_all[p, g, bj, r] = zt[g*64 + bj*16 + p%16]
            repp_cm = tc.tile_pool(name="repp", bufs=1)
            repp = repp_cm.__enter__()
            rep_all = repp.tile([128, G, 8, R], BF16, tag="rep_all", name="rep_all")
            ztr_all = repp.tile([128, G, 4, R], BF16, tag="ztr_all", name="ztr_all")
            zht, ztt = [], []
            for (w2tiles, phe, oh, bias, outl, tag) in (
                    (w2tt, pteT, oht, bts, ztt, "zt"),
                    (w2ht, pheT, ohh, bhs, zht, "zh")):
                with tc.tile_pool(name=f"ps7{tag}", bufs=1, space="PSUM") as ps7:
                    pps = [ps7.tile([128, R], F32, tag=f"{tag}ps{oc}",
                                    name=f"{tag}ps{oc}") for oc in range(6)]
                    for oc in range(6):
                        nc.tensor.matmul(pps[oc][:], phe[oc][:], oh[:],
                                         start=True, stop=False)
                        for dk in range(6):
                            nc.tensor.matmul(pps[oc][:],
                                             w2tiles[dk][:, oc * 128:(oc + 1) * 128],
                                             rst[dk][:], start=False, stop=(dk == 5))
                    for oc in range(6):
                        t = pp.tile([128, R], BF16, tag=f"{tag}{oc}", name=f"{tag}{oc}")
                        nc.scalar.activation(t[:], pps[oc][:],
                                             mybir.ActivationFunctionType.Tanh,
                                             bias=bias[:, oc:oc + 1])
                        outl.append(t)
                        # ph8 stage1 for this oc's two groups, then the
                        # group-pair's stage2 read immediately
                        gp = oc
                        if tag == "zh":
                            for gh in range(2):
                                g = oc * 2 + gh
                                dst = dataclasses.replace(
                                    XD[:], offset=XD[:].offset + g * 8 * R,
                                    ap=[[R, 8], [XDs, 8], [1, R]])
                                nc.sync.dma_start(dst, t[gh * 64:(gh + 1) * 64, :])
                                srcx = dataclasses.replace(
                                    XD[:], offset=XD[:].offset + g * 8 * R,
                                    ap=[[XDs, 8], [0, 16], [1, 8 * R]])
                                dstx = rep_all[:].rearrange("p g bi r -> p (g bi r)")
                                dstx = dstx[:, g * 8 * R:(g + 1) * 8 * R]
                                nc.sync.dma_start(dstx, srcx)
                        else:
                            for gh in range(2):
                                g = oc * 2 + gh
                                dst = dataclasses.replace(
                                    ZD[:], offset=ZD[:].offset + g * 4 * R,
                                    ap=[[ZDs, 16], [R, 4], [1, R]])
                                nc.sync.dma_start(dst, t[gh * 64:(gh + 1) * 64, :])
                            srcz = dataclasses.replace(
                                ZD[:], offset=ZD[:].offset + gp * 2 * 4 * R,
                                ap=[[0, 8], [ZDs, 16], [1, 2 * 4 * R]])
                            dstz = ztr_all[:].rearrange("p g bj r -> p (g bj r)")
                            dstz = dstz[:, gp * 2 * 4 * R:(gp + 1) * 2 * 4 * R]
                            nc.sync.dma_start(dstz, srcz)

            # ---- ph9: bilinear logits^T = sum_k WbT[k,:]^T * bl^T[k,:]
            # remaining WBT loads on sync (blocking on buffer reuse is fine
            # there: nothing queued behind except the final OUT store)
            for i in range(8, 12):
                t = wbp.tile([128, 32, NL], BF16, tag="wbc", name=f"wbc{i}")
                nc.sync.dma_start(t[:], WBT[:, i, :, :])
                wbts.append(t)

            with (
                tc.tile_pool(name="ps9", bufs=1, space="PSUM") as ps9,
            ):
                lt = ps9.tile([NL, R], F32, tag="lt_ps", name="lt_ps")
                ci = 0
                u = 0
                for g in range(G):
                    for bi in range(8):
                        blt = bltp.tile([128, 4, R], BF16, tag="blt", name="blt")
                        rep_b = _bcast(rep_all[:, g, bi, :], 4)
                        if u % PH9_POOL_MOD == PH9_POOL_MOD - 1:
                            nc.gpsimd.tensor_tensor(
                                blt[:], rep_b, ztr_all[:, g, :, :],
                                mybir.AluOpType.mult)
                        else:
                            nc.vector.tensor_tensor(
                                blt[:], rep_b, ztr_all[:, g, :, :],
                                mybir.AluOpType.mult)
                        u += 1
                        for bj in range(4):
                            nc.tensor.matmul(lt[:], wbts[ci // 32][:, ci % 32, :],
                                             blt[:, bj, :],
                                             start=(ci == 0), stop=(ci == 383))
                            ci += 1

                lout = pp.tile([NL, R], F32, tag="lout", name="lout")
                nc.vector.tensor_scalar_add(lout[:], lt[:], bbs[:, 0:1])
                nc.sync.dma_start(OUT[:], lout[:])
            repp_cm.__exit__(None, None, None)

    nc.finalize()
    return nc


def _t_perm():
    """Store-row permutation for the zt side: store q=g*64+j*4+bj holds
    logical o=g*64+bj*16+j."""
    perm = np.empty(H, np.int64)
    for g in range(G):
        for j in range(16):
            for bj in range(4):
                perm[g * 64 + j * 4 + bj] = g * 64 + bj * 16 + j
    return perm


def _wbt_perm(Wb):
    """WBT[p, load, slot, n] = Wb.T[k, n] for chunk ci=load*16+slot,
    k = g*4096 + i*64 + j, i = bi*8 + p//16, j = bj*16 + p%16,
    ci = g*32 + bi*4 + bj."""
    wbt = Wb.T  # [K, NL]
    p = np.arange(128)
    ip, jp = p // 16, p % 16
    rows = np.empty((384, 128), np.int64)
    for g in range(G):
        for bi in range(8):
            for bj in range(4):
                ci = g * 32 + bi * 4 + bj
                rows[ci] = g * 4096 + (bi * 8 + ip) * 64 + (bj * 16 + jp)
    w = wbt[rows]                                # [384, 128, NL]
    return np.ascontiguousarray(
        w.reshape(12, 32, 128, NL).transpose(2, 0, 1, 3)).astype(BF)


_CACHE = {}


def _prep_core_inputs(c, sequence_output, attention, mention_mask, Wh, bh, Wt, bt,
                      Wb, bb, mention_idx, hts):
    b, half = c // 2, c % 2
    seq_b = np.ascontiguousarray(sequence_output[b])              # [L, H]
    idx = mention_idx[b].astype(np.int64).reshape(EM)             # [96]
    mask = mention_mask[b].astype(np.float32)                     # [E, M]
    denom = mask.sum(-1)                                          # [E]

    emg = np.ascontiguousarray(seq_b[idx])                        # [96, H]
    amg = np.ascontiguousarray(
        attention[b][:, idx, :].transpose(1, 0, 2).reshape(EM, HL))

    sume = np.zeros((EM, E), np.float32)
    for e in range(E):
        for m in range(M):
            sume[e * M + m, e] = mask[e, m]

    hts_c = hts[b, half * R:(half + 1) * R].astype(np.int64)      # [R, 2]
    s = 1.0 / np.sqrt(np.float32(NH))
    wm = (mask / denom[:, None] * s).reshape(EM)                  # [96]
    whh = wm[:, None] * (hts_c[None, :, 0] == (np.arange(EM) // M)[:, None])
    wtt = wm[:, None] * (hts_c[None, :, 1] == (np.arange(EM) // M)[:, None])
    ohh = (hts_c[None, :, 0] == np.arange(E)[:, None]).astype(np.float32)
    oht = (hts_c[None, :, 1] == np.arange(E)[:, None]).astype(np.float32)

    tp = _CACHE.setdefault("tperm", _t_perm())
    w1h = np.ascontiguousarray(Wh[:, :H].T)                       # [768, 768]
    w2h = np.ascontiguousarray(Wh[:, H:].T)
    w1t = np.ascontiguousarray(Wt[tp, :H].T)
    w2t = np.ascontiguousarray(Wt[tp, H:].T)

    if "wbt" not in _CACHE or _CACHE.get("wbt_id") != id(Wb):
        _CACHE["wbt"] = _wbt_perm(Wb)
        _CACHE["wbt_id"] = id(Wb)

    return {
        "EMG": emg.astype(np.float32),
        "SUME": sume.astype(BF),
        "AMG": amg.astype(BF),
        "WHH": whh.astype(BF), "WTT": wtt.astype(BF),
        "OHH": ohh.astype(BF), "OHT": oht.astype(BF),
        "SEQ": seq_b.astype(BF),
        "W1H": w1h.astype(BF), "W2H": w2h.astype(BF),
        "W1T": w1t.astype(BF), "W2T": w2t.astype(BF),
        "WBT": _CACHE["wbt"],
        "BHS": np.ascontiguousarray(bh.reshape(6, 128).T).astype(np.float32),
        "BTS": np.ascontiguousarray(bt[tp].reshape(6, 128).T).astype(np.float32),
        "BBS": bb.reshape(NL, 1).astype(np.float32),
        "XD": np.zeros((8, G, 8, R), BF),
        "ZD": np.zeros((16, G, 4, R), BF),
    }


def kernel(sequence_output, attention, mention_mask, Wh, bh, Wt, bt, Wb, bb,
           mention_idx, hts):
    if "nc" not in _CACHE:
        _CACHE["nc"] = _build_program()
    nc = _CACHE["nc"]

    args = (np.asarray(sequence_output, np.float32), np.asarray(attention, np.float32),
            np.asarray(mention_mask, np.float32), np.asarray(Wh, np.float32),
            np.asarray(bh, np.float32), np.asarray(Wt, np.float32),
            np.asarray(bt, np.float32), np.asarray(Wb, np.float32),
            np.asarray(bb, np.float32), np.asarray(mention_idx),
            np.asarray(hts))
    in_maps = [_prep_core_inputs(c, *args) for c in range(8)]
    try:
        res = run_bass_kernel_spmd(nc, in_maps, list(range(8))).results
    except Exception:
        # transient NRT_EXEC_UNIT_UNRECOVERABLE has been observed on the
        # first execution of a freshly loaded NEFF; retry once
        res = run_bass_kernel_spmd(nc, in_maps, list(range(8))).results

    out = np.empty((B, P, NL), np.float32)
    for c in range(8):
        b, half = c // 2, c % 2
        out[b, half * R:(half + 1) * R, :] = np.asarray(res[c]["OUT"]).T
    return out


# revision 43
# speedup vs baseline: 1.5885x; 1.0809x over previous
"""Bass/Trainium2 kernel for DocRE bilinear segment-reduce model (v2).

Shapes (hardcoded): B=4, L=1024, H=768, NH=12, E=24, M=4, P=552, NL=97, BLK=64.
Sharding: 8 cores = (batch b = core//2) x (half of the 552 head-tail pairs).
Host prep is index-only (gathers/one-hots/permutations); all value compute
runs on device. TimelineSim-guided design; key structure:

- ph1: entity embeddings produced directly transposed (eetT [128d, 24e] x 6).
- ph2 folded into ph4: host combines the mention-mask weights with the pair
  one-hots (WHH/WTT [96, R]) so the entity-attention gathers contract K=96
  in one matmul per (head, l-chunk); no separate entity-attention pass.
- ph4: per l-chunk q, 6 head-pair rounds; t-side evacuated to SBUF bf16 by
  ACT; h-side multiplied from PSUM fp32 on DVE ('a') or fully evacuated and
  multiplied on DVE 2x ('b') / Pool ('c') per PH4_MIX; bf16 tree-sum.
- ph7a: project the 24 entity embeddings (not the 552 gathered pairs), then
  gather per pair with a K=24 matmul inside the ph7b PSUM accumulation.
- ph8: zh/zt replication tiles for the bilinear are materialized in SBUF
  bf16 via a DRAM round-trip (SBUF->DRAM reorder write, then DRAM->SBUF
  broadcast read with a step-0 mid-dim); SBUF-side partition-crossing DMAs
  are unreliable on HW, DRAM-side flat APs are exact.
- ph9: bilinear chunk layout (8 i's x 16 j's per 128-partition K-chunk):
  blt[p] = zh[g*64+bi*8+p//16] * zt[g*64+bj*16+p%16]; multiplies are
  all-bf16 DVE 2x_1p wide ops ([128, 4, 276], one per (g, bi), step-0
  broadcast on the rep operand), ~1/5 on Pool; accumulate = 384 K=128
  matmuls into one PSUM bank against host-permuted Wb chunks.
- Projections+ph8+ph9 run in two oc-halves so the second half's matmuls
  and DMAs overlap the first half's bilinear.
"""

import dataclasses
import numpy as np
import ml_dtypes

import concourse.bass as bass
import concourse.bacc as bacc
import concourse.tile as tile
from concourse import mybir
from concourse.bass_utils import run_bass_kernel_spmd

B, L, H, NH, E, M, P, NL, BLK = 4, 1024, 768, 12, 24, 4, 552, 97, 64
G = H // BLK            # 12 groups
R = P // 2              # 276 rows per core
EM = E * M              # 96 gathered mentions
HL = NH * L             # 12288
F32 = mybir.dt.float32
BF16 = mybir.dt.bfloat16
BF = ml_dtypes.bfloat16

# ph4 per-q engine mix for the 6 head-pair rounds:
#   'a' = t-evac only, DVE multiplies from PSUM fp32 (ACT 0.6us, DVE 1.0us)
#   'b' = full evac, DVE bf16 2x multiply     (ACT 1.2us, DVE 0.35us)
#   'c' = full evac, Pool bf16 multiply       (ACT 1.2us, Pool 1.2us)
PH4_MIX = ['a', 'a', 'a', 'a', 'c', 'c']
# ph9: units u with u % MOD == MOD-1 multiply on Pool, rest on DVE.
PH9_POOL_MOD = 5        # 19 of 96 units -> Pool


def _bcast(ap, n):
    """Insert a step-0 dim after the partition dim: [p, F] -> [p, n, F]."""
    return dataclasses.replace(ap, ap=[ap.ap[0], [0, n]] + ap.ap[1:])


def _build_program():
    nc = bacc.Bacc("TRN2", target_bir_lowering=False, debug=False, num_devices=8)
    dp = nc.declare_dram_parameter
    EMG = dp("EMG", [EM, H], F32, isOutput=False)       # gathered mention embeds
    SUME = dp("SUME", [EM, E], BF16, isOutput=False)    # mask one-hot
    AMG = dp("AMG", [EM, HL], BF16, isOutput=False)     # gathered attn rows (h-major)
    WHH = dp("WHH", [EM, R], BF16, isOutput=False)      # mask/denom/sqrtNH x head one-hot
    WTT = dp("WTT", [EM, R], BF16, isOutput=False)
    OHH = dp("OHH", [E, R], BF16, isOutput=False)       # head-entity one-hot
    OHT = dp("OHT", [E, R], BF16, isOutput=False)
    SEQ = dp("SEQ", [L, H], BF16, isOutput=False)
    W1H = dp("W1H", [H, H], BF16, isOutput=False)       # Wh[:, :768].T
    W2H = dp("W2H", [H, H], BF16, isOutput=False)       # Wh[:, 768:].T
    W1T = dp("W1T", [H, H], BF16, isOutput=False)       # Wt[perm, :768].T
    W2T = dp("W2T", [H, H], BF16, isOutput=False)
    WBT = dp("WBT", [128, 12, 32, NL], BF16, isOutput=False)  # permuted Wb.T
    BHS = dp("BHS", [128, 6], F32, isOutput=False)
    BTS = dp("BTS", [128, 6], F32, isOutput=False)
    BBS = dp("BBS", [NL, 1], F32, isOutput=False)
    XD = dp("XD", [8, G, 8, R], BF16, isOutput=False)   # scratch: zh reorder
    ZD = dp("ZD", [16, G, 4, R], BF16, isOutput=False)  # scratch: zt reorder
    OUT = dp("OUT", [NL, R], F32, isOutput=True)        # logits^T

    XDs, ZDs = 26496, 13248  # per-i / per-j row sizes (G*8*R, G*4*R)

    with tile.TileContext(nc) as tc:
        with (
            tc.tile_pool(name="persist", bufs=1) as pp,
            tc.tile_pool(name="wbt", bufs=8) as wbp,
            tc.tile_pool(name="blt", bufs=10) as bltp,
        ):
            # ---- persistent small loads (sync queue, program order = priority)
            def load(name_ap, shape, tag, dt=F32, eng=nc.sync):
                t = pp.tile(shape, dt, tag=tag, name=tag)
                eng.dma_start(t[:], name_ap)
                return t

            emg = load(EMG[:], [EM, H], "emg")
            sume = load(SUME[:], [EM, E], "sume", BF16)
            # amgp opened before w1p (LIFO pool closing: w1p closes first)
            amgp_cm = tc.tile_pool(name="amgp", bufs=1)
            amgp = amgp_cm.__enter__()
            w1p_cm = tc.tile_pool(name="w1p", bufs=1)
            w1p = w1p_cm.__enter__()
            whh = load(WHH[:], [EM, R], "whh", BF16)
            wtt = load(WTT[:], [EM, R], "wtt", BF16)
            ohh = load(OHH[:], [E, R], "ohh", BF16)
            oht = load(OHT[:], [E, R], "oht", BF16)
            bhs = load(BHS[:], [128, 6], "bhs")
            bts = load(BTS[:], [128, 6], "bts")
            bbs = load(BBS[:], [NL, 1], "bbs")
            # attention rows, 4 heads per load (pool scoped to ph4)
            amgq = []
            for hq in range(3):
                t = amgp.tile([EM, 4, L], BF16, tag=f"amg{hq}", name=f"amg{hq}")
                nc.sync.dma_start(
                    t[:], AMG[:, hq * 4 * L:(hq + 1) * 4 * L].rearrange(
                        "m (h l) -> m h l", h=4))
                amgq.append(t)
            amgt = [amgq[hp // 2][:, (hp % 2) * 2:(hp % 2) * 2 + 2, :]
                    for hp in range(6)]

            def load_rows_at(dram, off, n, pool, tag):
                t = pool.tile([128, n, H], BF16, tag=tag, name=tag)
                base = dram[:]
                src = dataclasses.replace(
                    base, offset=base.offset + off * 128 * H,
                    ap=[[H, 128], [128 * H, n], [1, H]])
                nc.sync.dma_start(t[:], src)
                return t

            # W1 weights next: ph7a runs right after ph4
            w1tp = [load_rows_at(W1T, 3 * i, 3, w1p, f"w1t{i}") for i in range(2)]
            w1hp = [load_rows_at(W1H, 3 * i, 3, w1p, f"w1h{i}") for i in range(2)]
            w1tt = [w1tp[dk // 3][:, dk % 3, :] for dk in range(6)]
            w1ht = [w1hp[dk // 3][:, dk % 3, :] for dk in range(6)]



            seqp = [load_rows_at(SEQ, 4 * i, 4, pp, f"seq{i}") for i in range(2)]
            seqt = [seqp[q // 4][:, q % 4, :] for q in range(8)]
            w2tp = [load_rows_at(W2T, 3 * i, 3, pp, f"w2t{i}") for i in range(2)]
            w2hp = [load_rows_at(W2H, 3 * i, 3, pp, f"w2h{i}") for i in range(2)]
            w2tt = [w2tp[dk // 3][:, dk % 3, :] for dk in range(6)]
            w2ht = [w2hp[dk // 3][:, dk % 3, :] for dk in range(6)]
            onesb = pp.tile([128, 1], BF16, tag="onesb", name="onesb")
            nc.vector.memset(onesb[:], 1.0)
            ones1 = pp.tile([1, 128], F32, tag="ones1", name="ones1")
            nc.vector.memset(ones1[:], 1.0)

            # 8 WBT loads prefetched on sync after all early-phase loads:
            # their transfers fill the DMA-idle ph4 window.
            wbts = []
            for i in range(8):
                t = wbp.tile([128, 32, NL], BF16, tag="wbc", name=f"wbc{i}")
                nc.sync.dma_start(t[:], WBT[:, i, :, :])
                wbts.append(t)

            # ---- ph1: eetT[d, e] = ln(sum_m sume[m, e] * exp(emg[m, d]))
            expt = pp.tile([EM, H], BF16, tag="expt", name="expt")
            nc.scalar.activation(expt[:], emg[:], mybir.ActivationFunctionType.Exp)
            eetT = []
            with tc.tile_pool(name="ps1", bufs=2, space="PSUM") as ps1:
                for dc in range(6):
                    pe = ps1.tile([128, E], F32, tag="ee_ps", name="ee_ps")
                    nc.tensor.matmul(pe[:], expt[:, dc * 128:(dc + 1) * 128],
                                     sume[:], start=True, stop=True)
                    t = pp.tile([128, E], BF16, tag=f"eetT{dc}", name=f"eetT{dc}")
                    nc.scalar.activation(t[:], pe[:], mybir.ActivationFunctionType.Ln)
                    eetT.append(t)

            # ---- ph4: ht_att accumulation per l-chunk q
            # hA/tA gathers K=96 (ph2 folded into WHH/WTT), 2 heads per round.
            htacc = []
            evp_cm = tc.tile_pool(name="evac", bufs=6)
            evp = evp_cm.__enter__()
            with (
                tc.tile_pool(name="ps4", bufs=2, space="PSUM") as ps4,
                tc.tile_pool(name="w4p", bufs=2) as w4p,
            ):
                def emit_tree(w4, q):
                    acc = pp.tile([128, R], BF16, tag=f"ht{q}", name=f"ht{q}")
                    nc.vector.tensor_add(w4[:, 0:6, :], w4[:, 0:6, :],
                                         w4[:, 6:12, :])
                    nc.gpsimd.tensor_add(w4[:, 0:3, :], w4[:, 0:3, :],
                                         w4[:, 3:6, :])
                    nc.vector.tensor_add(w4[:, 0, :], w4[:, 0, :], w4[:, 1, :])
                    nc.vector.tensor_add(acc[:], w4[:, 0, :], w4[:, 2, :])
                    htacc.append(acc)

                pending = None
                for q in range(8):
                    w4 = w4p.tile([128, NH, R], BF16, tag="w4", name="w4")
                    for hp in range(6):
                        if hp == 2 and pending is not None:
                            emit_tree(*pending)
                            pending = None
                        hh2 = ps4.tile([128, 2, 512], F32, tag="hh_ps", name="hh_ps")
                        tt2 = ps4.tile([128, 2, 512], F32, tag="tt_ps", name="tt_ps")
                        for kk in range(2):
                            amg_c = amgt[hp][:, kk, q * 128:(q + 1) * 128]
                            nc.tensor.matmul(hh2[:, kk, 0:R], amg_c, whh[:],
                                             start=True, stop=True)
                            nc.tensor.matmul(tt2[:, kk, 0:R], amg_c, wtt[:],
                                             start=True, stop=True)
                        # evacuate t-side (ACT), multiply per PH4_MIX
                        mode = PH4_MIX[hp]
                        tsb = evp.tile([128, 2, R], BF16, tag="tsb", name="tsb")
                        nc.scalar.copy(tsb[:], tt2[:, :, 0:R])
                        if mode == 'a':
                            nc.vector.tensor_tensor(
                                w4[:, hp * 2:hp * 2 + 2, :], hh2[:, :, 0:R],
                                tsb[:], mybir.AluOpType.mult)
                        else:
                            hsb = evp.tile([128, 2, R], BF16, tag="hsb", name="hsb")
                            nc.scalar.copy(hsb[:], hh2[:, :, 0:R])
                            eng = nc.vector if mode == 'b' else nc.gpsimd
                            eng.tensor_tensor(
                                w4[:, hp * 2:hp * 2 + 2, :], hsb[:], tsb[:],
                                mybir.AluOpType.mult)
                    # tree deferred into the next q's rounds (keeps the
                    # DVE queue from stalling on the Pool hop)
                    pending = (w4, q)
                emit_tree(*pending)
            evp_cm.__exit__(None, None, None)

            # ---- ph7a: entity-side projections ph_eT[e, o] (h and t)
            pheT, pteT = [], []
            for (w1l, outl, tag) in ((w1tt, pteT, "pte"), (w1ht, pheT, "phe")):
                with tc.tile_pool(name=f"ps7a{tag}", bufs=1, space="PSUM") as ps7a:
                    pgs = [ps7a.tile([E, 128], F32, tag=f"{tag}ps{oc}",
                                     name=f"{tag}ps{oc}") for oc in range(6)]
                    for dk in range(6):
                        for oc in range(6):
                            nc.tensor.matmul(pgs[oc][:], eetT[dk][:],
                                             w1l[dk][:, oc * 128:(oc + 1) * 128],
                                             start=(dk == 0), stop=(dk == 5))
                    for oc in range(6):
                        t = pp.tile([E, 128], BF16, tag=f"{tag}{oc}", name=f"{tag}{oc}")
                        nc.scalar.copy(t[:], pgs[oc][:])
                        outl.append(t)
            w1p_cm.__exit__(None, None, None)

            w1p_cm.__exit__(None, None, None)
            amgp_cm.__exit__(None, None, None)

            # ---- ph5: invd = 1/(sum_l ht + 1e-5), broadcast to 128 partitions
            invd = pp.tile([128, R], F32, tag="invd", name="invd")
            with tc.tile_pool(name="ps5", bufs=1, space="PSUM") as ps5:
                psum_s = ps5.tile([1, R], F32, tag="s_ps", name="s_ps")
                for q in range(8):
                    nc.tensor.matmul(psum_s[:], onesb[:], htacc[q][:],
                                     start=(q == 0), stop=(q == 7))
                invd1 = pp.tile([1, R], F32, tag="invd1", name="invd1")
                nc.vector.tensor_scalar_add(invd1[:], psum_s[:], 1e-5)
                nc.vector.reciprocal(invd1[:], invd1[:])
                pb = ps5.tile([128, R], F32, tag="invd_ps", name="invd_ps")
                nc.tensor.matmul(pb[:], ones1[:], invd1[:], start=True, stop=True)
                nc.scalar.copy(invd[:], pb[:])


            # ---- ph6: rs^T chunks (normalization folded into evac)
            rst = []
            with tc.tile_pool(name="ps6", bufs=2, space="PSUM") as ps6:
                for dc in range(6):
                    pr = ps6.tile([128, R], F32, tag="rs_ps", name="rs_ps")
                    for q in range(8):
                        nc.tensor.matmul(pr[:], seqt[q][:, dc * 128:(dc + 1) * 128],
                                         htacc[q][:], start=(q == 0), stop=(q == 7))
                    t = pp.tile([128, R], BF16, tag=f"rs{dc}", name=f"rs{dc}")
                    nc.vector.tensor_mul(t[:], pr[:], invd[:])
                    rst.append(t)

            # ---- ph7b + ph8 + ph9, split in two oc-halves so the second
            # half's projections and DMAs overlap the first half's bilinear.
            repp_cms = [tc.tile_pool(name=f"repp{i}", bufs=1) for i in range(12)]
            repps = [cm.__enter__() for cm in repp_cms]
            rept2 = [repps[gp].tile([128, 2, 8, R], BF16, tag=f"rep{gp}",
                                    name=f"rep{gp}") for gp in range(6)]
            ztr2 = [repps[6 + gp].tile([128, 2, 4, R], BF16, tag=f"ztr{gp}",
                                       name=f"ztr{gp}") for gp in range(6)]
            zht, ztt = [[None] * 6 for _ in range(2)]

            def project_half(half):
                ocs = range(half * 3, half * 3 + 3)
                for (w2l, phe, oh, bias, outl, tag) in (
                        (w2tt, pteT, oht, bts, ztt, "zt"),
                        (w2ht, pheT, ohh, bhs, zht, "zh")):
                    with tc.tile_pool(name=f"ps7{tag}{half}", bufs=1,
                                      space="PSUM") as ps7:
                        pps = {}
                        for oc in ocs:
                            pps[oc] = ps7.tile([128, R], F32, tag=f"{tag}ps{oc}",
                                               name=f"{tag}ps{oc}")
                            nc.tensor.matmul(pps[oc][:], phe[oc][:], oh[:],
                                             start=True, stop=False)
                            for dk in range(6):
                                nc.tensor.matmul(
                                    pps[oc][:], w2l[dk][:, oc * 128:(oc + 1) * 128],
                                    rst[dk][:], start=False, stop=(dk == 5))
                        for oc in ocs:
                            t = pp.tile([128, R], BF16, tag=f"{tag}{oc}",
                                        name=f"{tag}{oc}")
                            nc.scalar.activation(
                                t[:], pps[oc][:], mybir.ActivationFunctionType.Tanh,
                                bias=bias[:, oc:oc + 1])
                            outl[oc] = t
                            if tag == "zt":
                                for gh in range(2):
                                    g = oc * 2 + gh
                                    dst = dataclasses.replace(
                                        ZD[:], offset=ZD[:].offset + g * 4 * R,
                                        ap=[[ZDs, 16], [R, 4], [1, R]])
                                    nc.sync.dma_start(
                                        dst, t[gh * 64:(gh + 1) * 64, :])
                                srcz = dataclasses.replace(
                                    ZD[:], offset=ZD[:].offset + oc * 2 * 4 * R,
                                    ap=[[0, 8], [ZDs, 16], [1, 2 * 4 * R]])
                                dstz = ztr2[oc][:].rearrange(
                                    "p g bj r -> p (g bj r)")
                                nc.sync.dma_start(dstz, srcz)
                            else:
                                for gh in range(2):
                                    g = oc * 2 + gh
                                    dst = dataclasses.replace(
                                        XD[:], offset=XD[:].offset + g * 8 * R,
                                        ap=[[R, 8], [XDs, 8], [1, R]])
                                    nc.sync.dma_start(
                                        dst, t[gh * 64:(gh + 1) * 64, :])
                                srcx = dataclasses.replace(
                                    XD[:], offset=XD[:].offset + oc * 2 * 8 * R,
                                    ap=[[XDs, 8], [0, 16], [1, 2 * 8 * R]])
                                dstx = rept2[oc][:].rearrange(
                                    "p g bi r -> p (g bi r)")
                                nc.sync.dma_start(dstx, srcx)

            with (
                tc.tile_pool(name="ps9", bufs=1, space="PSUM") as ps9,
            ):
                lt = ps9.tile([NL, R], F32, tag="lt_ps", name="lt_ps")
                state = {"ci": 0, "u": 0}

                def ph9_groups(gs):
                    for g in gs:
                        for bi in range(8):
                            blt = bltp.tile([128, 4, R], BF16, tag="blt",
                                            name="blt")
                            rep_b = _bcast(rept2[g // 2][:, g % 2, bi, :], 4)
                            ztr_v = ztr2[g // 2][:, g % 2, :, :]
                            if state["u"] % PH9_POOL_MOD == PH9_POOL_MOD - 1:
                                nc.gpsimd.tensor_tensor(
                                    blt[:], rep_b, ztr_v, mybir.AluOpType.mult)
                            else:
                                nc.vector.tensor_tensor(
                                    blt[:], rep_b, ztr_v, mybir.AluOpType.mult)
                            state["u"] += 1
                            for bj in range(4):
                                ci = state["ci"]
                                nc.tensor.matmul(
                                    lt[:], wbts[ci // 32][:, ci % 32, :],
                                    blt[:, bj, :],
                                    start=(ci == 0), stop=(ci == 383))
                                state["ci"] += 1

                project_half(0)
                ph9_groups(range(0, 4))
                project_half(1)
                for i in range(8, 12):
                    t = wbp.tile([128, 32, NL], BF16, tag="wbc", name=f"wbc{i}")
                    nc.sync.dma_start(t[:], WBT[:, i, :, :])
                    wbts.append(t)
                ph9_groups(range(4, 12))

                lout = pp.tile([NL, R], F32, tag="lout", name="lout")
                nc.vector.tensor_scalar_add(lout[:], lt[:], bbs[:, 0:1])
                nc.sync.dma_start(OUT[:], lout[:])
            for cm in reversed(repp_cms):
                cm.__exit__(None, None, None)

    nc.finalize()
    return nc


def _t_perm():
    """Store-row permutation for the zt side: store q=g*64+j*4+bj holds
    logical o=g*64+bj*16+j."""
    perm = np.empty(H, np.int64)
    for g in range(G):
        for j in range(16):
            for bj in range(4):
                perm[g * 64 + j * 4 + bj] = g * 64 + bj * 16 + j
    return perm


def _wbt_perm(Wb):
    """WBT[p, load, slot, n] = Wb.T[k, n] for chunk ci=load*16+slot,
    k = g*4096 + i*64 + j, i = bi*8 + p//16, j = bj*16 + p%16,
    ci = g*32 + bi*4 + bj."""
    wbt = Wb.T  # [K, NL]
    p = np.arange(128)
    ip, jp = p // 16, p % 16
    rows = np.empty((384, 128), np.int64)
    for g in range(G):
        for bi in range(8):
            for bj in range(4):
                ci = g * 32 + bi * 4 + bj
                rows[ci] = g * 4096 + (bi * 8 + ip) * 64 + (bj * 16 + jp)
    w = wbt[rows]                                # [384, 128, NL]
    return np.ascontiguousarray(
        w.reshape(12, 32, 128, NL).transpose(2, 0, 1, 3)).astype(BF)


_CACHE = {}


def _prep_core_inputs(c, sequence_output, attention, mention_mask, Wh, bh, Wt, bt,
                      Wb, bb, mention_idx, hts):
    b, half = c // 2, c % 2
    seq_b = np.ascontiguousarray(sequence_output[b])              # [L, H]
    idx = mention_idx[b].astype(np.int64).reshape(EM)             # [96]
    mask = mention_mask[b].astype(np.float32)                     # [E, M]
    denom = mask.sum(-1)                                          # [E]

    emg = np.ascontiguousarray(seq_b[idx])                        # [96, H]
    amg = np.ascontiguousarray(
        attention[b][:, idx, :].transpose(1, 0, 2).reshape(EM, HL))

    sume = np.zeros((EM, E), np.float32)
    for e in range(E):
        for m in range(M):
            sume[e * M + m, e] = mask[e, m]

    hts_c = hts[b, half * R:(half + 1) * R].astype(np.int64)      # [R, 2]
    s = 1.0 / np.sqrt(np.float32(NH))
    wm = (mask / denom[:, None] * s).reshape(EM)                  # [96]
    whh = wm[:, None] * (hts_c[None, :, 0] == (np.arange(EM) // M)[:, None])
    wtt = wm[:, None] * (hts_c[None, :, 1] == (np.arange(EM) // M)[:, None])
    ohh = (hts_c[None, :, 0] == np.arange(E)[:, None]).astype(np.float32)
    oht = (hts_c[None, :, 1] == np.arange(E)[:, None]).astype(np.float32)

    tp = _CACHE.setdefault("tperm", _t_perm())
    w1h = np.ascontiguousarray(Wh[:, :H].T)                       # [768, 768]
    w2h = np.ascontiguousarray(Wh[:, H:].T)
    w1t = np.ascontiguousarray(Wt[tp, :H].T)
    w2t = np.ascontiguousarray(Wt[tp, H:].T)

    if "wbt" not in _CACHE:
        _CACHE["wbt"] = _wbt_perm(Wb)

    return {
        "EMG": emg.astype(np.float32),
        "SUME": sume.astype(BF),
        "AMG": amg.astype(BF),
        "WHH": whh.astype(BF), "WTT": wtt.astype(BF),
        "OHH": ohh.astype(BF), "OHT": oht.astype(BF),
        "SEQ": seq_b.astype(BF),
        "W1H": w1h.astype(BF), "W2H": w2h.astype(BF),
        "W1T": w1t.astype(BF), "W2T": w2t.astype(BF),
        "WBT": _CACHE["wbt"],
        "BHS": np.ascontiguousarray(bh.reshape(6, 128).T).astype(np.float32),
        "BTS": np.ascontiguousarray(bt[tp].reshape(6, 128).T).astype(np.float32),
        "BBS": bb.reshape(NL, 1).astype(np.float32),
        "XD": np.zeros((8, G, 8, R), BF),
        "ZD": np.zeros((16, G, 4, R), BF),
    }


def kernel(sequence_output, attention, mention_mask, Wh, bh, Wt, bt, Wb, bb,
           mention_idx, hts):
    if "nc" not in _CACHE:
        _CACHE["nc"] = _build_program()
    nc = _CACHE["nc"]

    args = (np.asarray(sequence_output, np.float32), np.asarray(attention, np.float32),
            np.asarray(mention_mask, np.float32), np.asarray(Wh, np.float32),
            np.asarray(bh, np.float32), np.asarray(Wt, np.float32),
            np.asarray(bt, np.float32), np.asarray(Wb, np.float32),
            np.asarray(bb, np.float32), np.asarray(mention_idx),
            np.asarray(hts))
    _CACHE.pop("wbt", None)   # Wb may differ between calls
    in_maps = [_prep_core_inputs(c, *args) for c in range(8)]
    res = None
    for attempt in range(4):
        try:
            res = run_bass_kernel_spmd(nc, in_maps, list(range(8))).results
            break
        except Exception:
            # transient NRT_EXEC_UNIT_UNRECOVERABLE is occasionally raised by
            # the runtime; back off briefly and retry
            if attempt == 3:
                raise
            import time
            time.sleep(2.0 * (attempt + 1))

    out = np.empty((B, P, NL), np.float32)
    for c in range(8):
        b, half = c // 2, c % 2
        out[b, half * R:(half + 1) * R, :] = np.asarray(res[c]["OUT"]).T
    return out


# revision 44
# speedup vs baseline: 1.5916x; 1.0019x over previous
"""Bass/Trainium2 kernel for DocRE bilinear segment-reduce model (v2).

Shapes (hardcoded): B=4, L=1024, H=768, NH=12, E=24, M=4, P=552, NL=97, BLK=64.
Sharding: 8 cores = (batch b = core//2) x (half of the 552 head-tail pairs).
Host prep is index-only (gathers/one-hots/permutations); all value compute
runs on device. TimelineSim-guided design; key structure:

- ph1: entity embeddings produced directly transposed (eetT [128d, 24e] x 6).
- ph2 folded into ph4: host combines the mention-mask weights with the pair
  one-hots (WHH/WTT [96, R]) so the entity-attention gathers contract K=96
  in one matmul per (head, l-chunk); no separate entity-attention pass.
- ph4: per l-chunk q, 6 head-pair rounds; t-side evacuated to SBUF bf16 by
  ACT; h-side multiplied from PSUM fp32 on DVE ('a') or fully evacuated and
  multiplied on DVE 2x ('b') / Pool ('c') per PH4_MIX; bf16 tree-sum.
- ph7a: project the 24 entity embeddings (not the 552 gathered pairs), then
  gather per pair with a K=24 matmul inside the ph7b PSUM accumulation.
- ph8: zh/zt replication tiles for the bilinear are materialized in SBUF
  bf16 via a DRAM round-trip (SBUF->DRAM reorder write, then DRAM->SBUF
  broadcast read with a step-0 mid-dim); SBUF-side partition-crossing DMAs
  are unreliable on HW, DRAM-side flat APs are exact.
- ph9: bilinear chunk layout (8 i's x 16 j's per 128-partition K-chunk):
  blt[p] = zh[g*64+bi*8+p//16] * zt[g*64+bj*16+p%16]; multiplies are
  all-bf16 DVE 2x_1p wide ops ([128, 4, 276], one per (g, bi), step-0
  broadcast on the rep operand), ~1/5 on Pool; accumulate = 384 K=128
  matmuls into one PSUM bank against host-permuted Wb chunks.
- Projections+ph8+ph9 run in two oc-halves so the second half's matmuls
  and DMAs overlap the first half's bilinear.
"""

import dataclasses
import numpy as np
import ml_dtypes

import concourse.bass as bass
import concourse.bacc as bacc
import concourse.tile as tile
from concourse import mybir
from concourse.bass_utils import run_bass_kernel_spmd

B, L, H, NH, E, M, P, NL, BLK = 4, 1024, 768, 12, 24, 4, 552, 97, 64
G = H // BLK            # 12 groups
R = P // 2              # 276 rows per core
EM = E * M              # 96 gathered mentions
HL = NH * L             # 12288
F32 = mybir.dt.float32
BF16 = mybir.dt.bfloat16
BF = ml_dtypes.bfloat16

# ph4 per-q engine mix for the 6 head-pair rounds:
#   'a' = t-evac only, DVE multiplies from PSUM fp32 (ACT 0.6us, DVE 1.0us)
#   'b' = full evac, DVE bf16 2x multiply     (ACT 1.2us, DVE 0.35us)
#   'c' = full evac, Pool bf16 multiply       (ACT 1.2us, Pool 1.2us)
PH4_MIX = ['a', 'a', 'a', 'a', 'c', 'c']
# ph9: units u with u % MOD == MOD-1 multiply on Pool, rest on DVE.
PH9_POOL_MOD = 5        # 19 of 96 units -> Pool


def _bcast(ap, n):
    """Insert a step-0 dim after the partition dim: [p, F] -> [p, n, F]."""
    return dataclasses.replace(ap, ap=[ap.ap[0], [0, n]] + ap.ap[1:])


def _build_program():
    nc = bacc.Bacc("TRN2", target_bir_lowering=False, debug=False, num_devices=8)
    dp = nc.declare_dram_parameter
    EMG = dp("EMG", [EM, H], F32, isOutput=False)       # gathered mention embeds
    SUME = dp("SUME", [EM, E], BF16, isOutput=False)    # mask one-hot
    AMG = dp("AMG", [EM, HL], BF16, isOutput=False)     # gathered attn rows (h-major)
    WHH = dp("WHH", [EM, R], BF16, isOutput=False)      # mask/denom/sqrtNH x head one-hot
    WTT = dp("WTT", [EM, R], BF16, isOutput=False)
    OHH = dp("OHH", [E, R], BF16, isOutput=False)       # head-entity one-hot
    OHT = dp("OHT", [E, R], BF16, isOutput=False)
    SEQ = dp("SEQ", [L, H], BF16, isOutput=False)
    W1H = dp("W1H", [H, H], BF16, isOutput=False)       # Wh[:, :768].T
    W2H = dp("W2H", [H, H], BF16, isOutput=False)       # Wh[:, 768:].T
    W1T = dp("W1T", [H, H], BF16, isOutput=False)       # Wt[perm, :768].T
    W2T = dp("W2T", [H, H], BF16, isOutput=False)
    WBT = dp("WBT", [128, 12, 32, NL], BF16, isOutput=False)  # permuted Wb.T
    BHS = dp("BHS", [128, 6], F32, isOutput=False)
    BTS = dp("BTS", [128, 6], F32, isOutput=False)
    BBS = dp("BBS", [NL, 1], F32, isOutput=False)
    XD = dp("XD", [8, G, 8, R], BF16, isOutput=False)   # scratch: zh reorder
    ZD = dp("ZD", [16, G, 4, R], BF16, isOutput=False)  # scratch: zt reorder
    OUT = dp("OUT", [NL, R], F32, isOutput=True)        # logits^T

    XDs, ZDs = 26496, 13248  # per-i / per-j row sizes (G*8*R, G*4*R)

    with tile.TileContext(nc) as tc:
        with (
            tc.tile_pool(name="persist", bufs=1) as pp,
            tc.tile_pool(name="wbt", bufs=8) as wbp,
            tc.tile_pool(name="blt", bufs=10) as bltp,
        ):
            # ---- persistent small loads (sync queue, program order = priority)
            def load(name_ap, shape, tag, dt=F32, eng=nc.sync):
                t = pp.tile(shape, dt, tag=tag, name=tag)
                eng.dma_start(t[:], name_ap)
                return t

            emg = load(EMG[:], [EM, H], "emg")
            sume = load(SUME[:], [EM, E], "sume", BF16)
            # amgp opened before w1p (LIFO pool closing: w1p closes first)
            amgp_cm = tc.tile_pool(name="amgp", bufs=1)
            amgp = amgp_cm.__enter__()
            w1p_cm = tc.tile_pool(name="w1p", bufs=1)
            w1p = w1p_cm.__enter__()
            whh = load(WHH[:], [EM, R], "whh", BF16)
            wtt = load(WTT[:], [EM, R], "wtt", BF16)
            ohh = load(OHH[:], [E, R], "ohh", BF16)
            oht = load(OHT[:], [E, R], "oht", BF16)
            bhs = load(BHS[:], [128, 6], "bhs")
            bts = load(BTS[:], [128, 6], "bts")
            bbs = load(BBS[:], [NL, 1], "bbs")
            # attention rows, 4 heads per load (pool scoped to ph4)
            amgq = []
            for hq in range(3):
                t = amgp.tile([EM, 4, L], BF16, tag=f"amg{hq}", name=f"amg{hq}")
                nc.sync.dma_start(
                    t[:], AMG[:, hq * 4 * L:(hq + 1) * 4 * L].rearrange(
                        "m (h l) -> m h l", h=4))
                amgq.append(t)
            amgt = [amgq[hp // 2][:, (hp % 2) * 2:(hp % 2) * 2 + 2, :]
                    for hp in range(6)]

            def load_rows_at(dram, off, n, pool, tag):
                t = pool.tile([128, n, H], BF16, tag=tag, name=tag)
                base = dram[:]
                src = dataclasses.replace(
                    base, offset=base.offset + off * 128 * H,
                    ap=[[H, 128], [128 * H, n], [1, H]])
                nc.sync.dma_start(t[:], src)
                return t

            # W1 weights next: ph7a runs right after ph4
            w1tp = [load_rows_at(W1T, 3 * i, 3, w1p, f"w1t{i}") for i in range(2)]
            w1hp = [load_rows_at(W1H, 3 * i, 3, w1p, f"w1h{i}") for i in range(2)]
            w1tt = [w1tp[dk // 3][:, dk % 3, :] for dk in range(6)]
            w1ht = [w1hp[dk // 3][:, dk % 3, :] for dk in range(6)]



            seqp = [load_rows_at(SEQ, 4 * i, 4, pp, f"seq{i}") for i in range(2)]
            seqt = [seqp[q // 4][:, q % 4, :] for q in range(8)]
            w2tp = [load_rows_at(W2T, 3 * i, 3, pp, f"w2t{i}") for i in range(2)]
            w2hp = [load_rows_at(W2H, 3 * i, 3, pp, f"w2h{i}") for i in range(2)]
            w2tt = [w2tp[dk // 3][:, dk % 3, :] for dk in range(6)]
            w2ht = [w2hp[dk // 3][:, dk % 3, :] for dk in range(6)]
            onesb = pp.tile([128, 1], BF16, tag="onesb", name="onesb")
            nc.vector.memset(onesb[:], 1.0)
            ones1 = pp.tile([1, 128], F32, tag="ones1", name="ones1")
            nc.vector.memset(ones1[:], 1.0)

            # 8 WBT loads prefetched on sync after all early-phase loads:
            # their transfers fill the DMA-idle ph4 window.
            wbts = []
            for i in range(8):
                t = wbp.tile([128, 32, NL], BF16, tag="wbc", name=f"wbc{i}")
                nc.sync.dma_start(t[:], WBT[:, i, :, :])
                wbts.append(t)

            # ---- ph1: eetT[d, e] = ln(sum_m sume[m, e] * exp(emg[m, d]))
            expt = pp.tile([EM, H], BF16, tag="expt", name="expt")
            nc.scalar.activation(expt[:], emg[:], mybir.ActivationFunctionType.Exp)
            eetT = []
            with tc.tile_pool(name="ps1", bufs=2, space="PSUM") as ps1:
                for dc in range(6):
                    pe = ps1.tile([128, E], F32, tag="ee_ps", name="ee_ps")
                    nc.tensor.matmul(pe[:], expt[:, dc * 128:(dc + 1) * 128],
                                     sume[:], start=True, stop=True)
                    t = pp.tile([128, E], BF16, tag=f"eetT{dc}", name=f"eetT{dc}")
                    nc.scalar.activation(t[:], pe[:], mybir.ActivationFunctionType.Ln)
                    eetT.append(t)

            # ---- ph4: ht_att accumulation per l-chunk q
            # hA/tA gathers K=96 (ph2 folded into WHH/WTT), 2 heads per round.
            htacc = []
            evp_cm = tc.tile_pool(name="evac", bufs=6)
            evp = evp_cm.__enter__()
            with (
                tc.tile_pool(name="ps4", bufs=2, space="PSUM") as ps4,
                tc.tile_pool(name="w4p", bufs=2) as w4p,
            ):
                def emit_tree(w4, q):
                    acc = pp.tile([128, R], BF16, tag=f"ht{q}", name=f"ht{q}")
                    nc.vector.tensor_add(w4[:, 0:6, :], w4[:, 0:6, :],
                                         w4[:, 6:12, :])
                    nc.gpsimd.tensor_add(w4[:, 0:3, :], w4[:, 0:3, :],
                                         w4[:, 3:6, :])
                    nc.vector.tensor_add(w4[:, 0, :], w4[:, 0, :], w4[:, 1, :])
                    nc.vector.tensor_add(acc[:], w4[:, 0, :], w4[:, 2, :])
                    htacc.append(acc)

                pending = None
                for q in range(8):
                    w4 = w4p.tile([128, NH, R], BF16, tag="w4", name="w4")
                    for hp in range(6):
                        if hp == 2 and pending is not None:
                            emit_tree(*pending)
                            pending = None
                        hh2 = ps4.tile([128, 2, 512], F32, tag="hh_ps", name="hh_ps")
                        tt2 = ps4.tile([128, 2, 512], F32, tag="tt_ps", name="tt_ps")
                        for kk in range(2):
                            amg_c = amgt[hp][:, kk, q * 128:(q + 1) * 128]
                            nc.tensor.matmul(hh2[:, kk, 0:R], amg_c, whh[:],
                                             start=True, stop=True)
                            nc.tensor.matmul(tt2[:, kk, 0:R], amg_c, wtt[:],
                                             start=True, stop=True)
                        # evacuate t-side (ACT), multiply per PH4_MIX
                        mode = PH4_MIX[hp]
                        tsb = evp.tile([128, 2, R], BF16, tag="tsb", name="tsb")
                        nc.scalar.copy(tsb[:], tt2[:, :, 0:R])
                        if mode == 'a':
                            nc.vector.tensor_tensor(
                                w4[:, hp * 2:hp * 2 + 2, :], hh2[:, :, 0:R],
                                tsb[:], mybir.AluOpType.mult)
                        else:
                            hsb = evp.tile([128, 2, R], BF16, tag="hsb", name="hsb")
                            nc.scalar.copy(hsb[:], hh2[:, :, 0:R])
                            eng = nc.vector if mode == 'b' else nc.gpsimd
                            eng.tensor_tensor(
                                w4[:, hp * 2:hp * 2 + 2, :], hsb[:], tsb[:],
                                mybir.AluOpType.mult)
                    # tree deferred into the next q's rounds (keeps the
                    # DVE queue from stalling on the Pool hop)
                    pending = (w4, q)
                emit_tree(*pending)
            evp_cm.__exit__(None, None, None)

            # ---- ph7a: entity-side projections ph_eT[e, o] (h and t)
            pheT, pteT = [], []
            for (w1l, outl, tag) in ((w1tt, pteT, "pte"), (w1ht, pheT, "phe")):
                with tc.tile_pool(name=f"ps7a{tag}", bufs=1, space="PSUM") as ps7a:
                    pgs = [ps7a.tile([E, 128], F32, tag=f"{tag}ps{oc}",
                                     name=f"{tag}ps{oc}") for oc in range(6)]
                    for dk in range(6):
                        for oc in range(6):
                            nc.tensor.matmul(pgs[oc][:], eetT[dk][:],
                                             w1l[dk][:, oc * 128:(oc + 1) * 128],
                                             start=(dk == 0), stop=(dk == 5))
                    for oc in range(6):
                        t = pp.tile([E, 128], BF16, tag=f"{tag}{oc}", name=f"{tag}{oc}")
                        nc.scalar.copy(t[:], pgs[oc][:])
                        outl.append(t)
            w1p_cm.__exit__(None, None, None)

            w1p_cm.__exit__(None, None, None)
            amgp_cm.__exit__(None, None, None)

            # ---- ph5: invd = 1/(sum_l ht + 1e-5), broadcast to 128 partitions
            invd = pp.tile([128, R], F32, tag="invd", name="invd")
            with tc.tile_pool(name="ps5", bufs=1, space="PSUM") as ps5:
                psum_s = ps5.tile([1, R], F32, tag="s_ps", name="s_ps")
                for q in range(8):
                    nc.tensor.matmul(psum_s[:], onesb[:], htacc[q][:],
                                     start=(q == 0), stop=(q == 7))
                invd1 = pp.tile([1, R], F32, tag="invd1", name="invd1")
                nc.vector.tensor_scalar_add(invd1[:], psum_s[:], 1e-5)
                nc.vector.reciprocal(invd1[:], invd1[:])
                pb = ps5.tile([128, R], F32, tag="invd_ps", name="invd_ps")
                nc.tensor.matmul(pb[:], ones1[:], invd1[:], start=True, stop=True)
                nc.scalar.copy(invd[:], pb[:])


            # ---- ph6: rs^T chunks (normalization folded into evac)
            rst = []
            with tc.tile_pool(name="ps6", bufs=2, space="PSUM") as ps6:
                for dc in range(6):
                    pr = ps6.tile([128, R], F32, tag="rs_ps", name="rs_ps")
                    for q in range(8):
                        nc.tensor.matmul(pr[:], seqt[q][:, dc * 128:(dc + 1) * 128],
                                         htacc[q][:], start=(q == 0), stop=(q == 7))
                    t = pp.tile([128, R], BF16, tag=f"rs{dc}", name=f"rs{dc}")
                    nc.vector.tensor_mul(t[:], pr[:], invd[:])
                    rst.append(t)

            # ---- ph7b + ph8 + ph9, split in two oc-halves so the second
            # half's projections and DMAs overlap the first half's bilinear.
            repp_cms = [tc.tile_pool(name=f"repp{i}", bufs=1) for i in range(12)]
            repps = [cm.__enter__() for cm in repp_cms]
            rept2 = [repps[gp].tile([128, 2, 8, R], BF16, tag=f"rep{gp}",
                                    name=f"rep{gp}") for gp in range(6)]
            ztr2 = [repps[6 + gp].tile([128, 2, 4, R], BF16, tag=f"ztr{gp}",
                                       name=f"ztr{gp}") for gp in range(6)]
            zht, ztt = [[None] * 6 for _ in range(2)]

            def project_half(half):
                ocs = range(half * 3, half * 3 + 3)
                for (w2l, phe, oh, bias, outl, tag) in (
                        (w2tt, pteT, oht, bts, ztt, "zt"),
                        (w2ht, pheT, ohh, bhs, zht, "zh")):
                    with tc.tile_pool(name=f"ps7{tag}{half}", bufs=1,
                                      space="PSUM") as ps7:
                        pps = {}
                        for oc in ocs:
                            pps[oc] = ps7.tile([128, R], F32, tag=f"{tag}ps{oc}",
                                               name=f"{tag}ps{oc}")
                            nc.tensor.matmul(pps[oc][:], phe[oc][:], oh[:],
                                             start=True, stop=False)
                            for dk in range(6):
                                nc.tensor.matmul(
                                    pps[oc][:], w2l[dk][:, oc * 128:(oc + 1) * 128],
                                    rst[dk][:], start=False, stop=(dk == 5))
                        for oc in ocs:
                            t = pp.tile([128, R], BF16, tag=f"{tag}{oc}",
                                        name=f"{tag}{oc}")
                            nc.scalar.activation(
                                t[:], pps[oc][:], mybir.ActivationFunctionType.Tanh,
                                bias=bias[:, oc:oc + 1])
                            outl[oc] = t
                            if tag == "zt":
                                for gh in range(2):
                                    g = oc * 2 + gh
                                    dst = dataclasses.replace(
                                        ZD[:], offset=ZD[:].offset + g * 4 * R,
                                        ap=[[ZDs, 16], [R, 4], [1, R]])
                                    nc.sync.dma_start(
                                        dst, t[gh * 64:(gh + 1) * 64, :])
                                srcz = dataclasses.replace(
                                    ZD[:], offset=ZD[:].offset + oc * 2 * 4 * R,
                                    ap=[[0, 8], [ZDs, 16], [1, 2 * 4 * R]])
                                dstz = ztr2[oc][:].rearrange(
                                    "p g bj r -> p (g bj r)")
                                nc.sync.dma_start(dstz, srcz)
                            else:
                                for gh in range(2):
                                    g = oc * 2 + gh
                                    dst = dataclasses.replace(
                                        XD[:], offset=XD[:].offset + g * 8 * R,
                                        ap=[[R, 8], [XDs, 8], [1, R]])
                                    nc.sync.dma_start(
                                        dst, t[gh * 64:(gh + 1) * 64, :])
                                srcx = dataclasses.replace(
                                    XD[:], offset=XD[:].offset + oc * 2 * 8 * R,
                                    ap=[[XDs, 8], [0, 16], [1, 2 * 8 * R]])
                                dstx = rept2[oc][:].rearrange(
                                    "p g bi r -> p (g bi r)")
                                nc.sync.dma_start(dstx, srcx)

            with (
                tc.tile_pool(name="ps9", bufs=1, space="PSUM") as ps9,
            ):
                lt = ps9.tile([NL, R], F32, tag="lt_ps", name="lt_ps")
                state = {"ci": 0, "u": 0}

                def ph9_groups(gs):
                    for g in gs:
                        for bi in range(8):
                            blt = bltp.tile([128, 4, R], BF16, tag="blt",
                                            name="blt")
                            rep_b = _bcast(rept2[g // 2][:, g % 2, bi, :], 4)
                            ztr_v = ztr2[g // 2][:, g % 2, :, :]
                            if state["u"] % PH9_POOL_MOD == 2:
                                nc.gpsimd.tensor_tensor(
                                    blt[:], rep_b, ztr_v, mybir.AluOpType.mult)
                            else:
                                nc.vector.tensor_tensor(
                                    blt[:], rep_b, ztr_v, mybir.AluOpType.mult)
                            state["u"] += 1
                            for bj in range(4):
                                ci = state["ci"]
                                nc.tensor.matmul(
                                    lt[:], wbts[ci // 32][:, ci % 32, :],
                                    blt[:, bj, :],
                                    start=(ci == 0), stop=(ci == 383))
                                state["ci"] += 1

                project_half(0)
                ph9_groups(range(0, 4))
                project_half(1)
                for i in range(8, 12):
                    t = wbp.tile([128, 32, NL], BF16, tag="wbc", name=f"wbc{i}")
                    nc.sync.dma_start(t[:], WBT[:, i, :, :])
                    wbts.append(t)
                ph9_groups(range(4, 12))

                lout = pp.tile([NL, R], F32, tag="lout", name="lout")
                nc.vector.tensor_scalar_add(lout[:], lt[:], bbs[:, 0:1])
                nc.sync.dma_start(OUT[:], lout[:])
            for cm in reversed(repp_cms):
                cm.__exit__(None, None, None)

    nc.finalize()
    return nc


def _t_perm():
    """Store-row permutation for the zt side: store q=g*64+j*4+bj holds
    logical o=g*64+bj*16+j."""
    perm = np.empty(H, np.int64)
    for g in range(G):
        for j in range(16):
            for bj in range(4):
                perm[g * 64 + j * 4 + bj] = g * 64 + bj * 16 + j
    return perm


def _wbt_perm(Wb):
    """WBT[p, load, slot, n] = Wb.T[k, n] for chunk ci=load*16+slot,
    k = g*4096 + i*64 + j, i = bi*8 + p//16, j = bj*16 + p%16,
    ci = g*32 + bi*4 + bj."""
    wbt = Wb.T  # [K, NL]
    p = np.arange(128)
    ip, jp = p // 16, p % 16
    rows = np.empty((384, 128), np.int64)
    for g in range(G):
        for bi in range(8):
            for bj in range(4):
                ci = g * 32 + bi * 4 + bj
                rows[ci] = g * 4096 + (bi * 8 + ip) * 64 + (bj * 16 + jp)
    w = wbt[rows]                                # [384, 128, NL]
    return np.ascontiguousarray(
        w.reshape(12, 32, 128, NL).transpose(2, 0, 1, 3)).astype(BF)


_CACHE = {}


def _prep_core_inputs(c, sequence_output, attention, mention_mask, Wh, bh, Wt, bt,
                      Wb, bb, mention_idx, hts):
    b, half = c // 2, c % 2
    seq_b = np.ascontiguousarray(sequence_output[b])              # [L, H]
    idx = mention_idx[b].astype(np.int64).reshape(EM)             # [96]
    mask = mention_mask[b].astype(np.float32)                     # [E, M]
    denom = mask.sum(-1)                                          # [E]

    emg = np.ascontiguousarray(seq_b[idx])                        # [96, H]
    amg = np.ascontiguousarray(
        attention[b][:, idx, :].transpose(1, 0, 2).reshape(EM, HL))

    sume = np.zeros((EM, E), np.float32)
    for e in range(E):
        for m in range(M):
            sume[e * M + m, e] = mask[e, m]

    hts_c = hts[b, half * R:(half + 1) * R].astype(np.int64)      # [R, 2]
    s = 1.0 / np.sqrt(np.float32(NH))
    wm = (mask / denom[:, None] * s).reshape(EM)                  # [96]
    whh = wm[:, None] * (hts_c[None, :, 0] == (np.arange(EM) // M)[:, None])
    wtt = wm[:, None] * (hts_c[None, :, 1] == (np.arange(EM) // M)[:, None])
    ohh = (hts_c[None, :, 0] == np.arange(E)[:, None]).astype(np.float32)
    oht = (hts_c[None, :, 1] == np.arange(E)[:, None]).astype(np.float32)

    tp = _CACHE.setdefault("tperm", _t_perm())
    w1h = np.ascontiguousarray(Wh[:, :H].T)                       # [768, 768]
    w2h = np.ascontiguousarray(Wh[:, H:].T)
    w1t = np.ascontiguousarray(Wt[tp, :H].T)
    w2t = np.ascontiguousarray(Wt[tp, H:].T)

    if "wbt" not in _CACHE:
        _CACHE["wbt"] = _wbt_perm(Wb)

    return {
        "EMG": emg.astype(np.float32),
        "SUME": sume.astype(BF),
        "AMG": amg.astype(BF),
        "WHH": whh.astype(BF), "WTT": wtt.astype(BF),
        "OHH": ohh.astype(BF), "OHT": oht.astype(BF),
        "SEQ": seq_b.astype(BF),
        "W1H": w1h.astype(BF), "W2H": w2h.astype(BF),
        "W1T": w1t.astype(BF), "W2T": w2t.astype(BF),
        "WBT": _CACHE["wbt"],
        "BHS": np.ascontiguousarray(bh.reshape(6, 128).T).astype(np.float32),
        "BTS": np.ascontiguousarray(bt[tp].reshape(6, 128).T).astype(np.float32),
        "BBS": bb.reshape(NL, 1).astype(np.float32),
        "XD": np.zeros((8, G, 8, R), BF),
        "ZD": np.zeros((16, G, 4, R), BF),
    }


def kernel(sequence_output, attention, mention_mask, Wh, bh, Wt, bt, Wb, bb,
           mention_idx, hts):
    if "nc" not in _CACHE:
        _CACHE["nc"] = _build_program()
    nc = _CACHE["nc"]

    args = (np.asarray(sequence_output, np.float32), np.asarray(attention, np.float32),
            np.asarray(mention_mask, np.float32), np.asarray(Wh, np.float32),
            np.asarray(bh, np.float32), np.asarray(Wt, np.float32),
            np.asarray(bt, np.float32), np.asarray(Wb, np.float32),
            np.asarray(bb, np.float32), np.asarray(mention_idx),
            np.asarray(hts))
    _CACHE.pop("wbt", None)   # Wb may differ between calls
    in_maps = [_prep_core_inputs(c, *args) for c in range(8)]
    res = None
    for attempt in range(4):
        try:
            res = run_bass_kernel_spmd(nc, in_maps, list(range(8))).results
            break
        except Exception:
            # transient NRT_EXEC_UNIT_UNRECOVERABLE is occasionally raised by
            # the runtime; back off briefly and retry
            if attempt == 3:
                raise
            import time
            time.sleep(2.0 * (attempt + 1))

    out = np.empty((B, P, NL), np.float32)
    for c in range(8):
        b, half = c // 2, c % 2
        out[b, half * R:(half + 1) * R, :] = np.asarray(res[c]["OUT"]).T
    return out


# revision 45
# speedup vs baseline: 1.6660x; 1.0468x over previous
"""Bass/Trainium2 kernel for DocRE bilinear segment-reduce model (v2).

Shapes (hardcoded): B=4, L=1024, H=768, NH=12, E=24, M=4, P=552, NL=97, BLK=64.
Sharding: 8 cores = (batch b = core//2) x (half of the 552 head-tail pairs).
Host prep is index-only (gathers/one-hots/permutations); all value compute
runs on device. TimelineSim-guided design; key structure:

- ph1: entity embeddings produced directly transposed (eetT [128d, 24e] x 6).
- ph2 folded into ph4: host combines the mention-mask weights with the pair
  one-hots (WHH/WTT [96, R]) so the entity-attention gathers contract K=96
  in one matmul per (head, l-chunk); no separate entity-attention pass.
- ph4: per l-chunk q, 6 head-pair rounds; t-side evacuated to SBUF bf16 by
  ACT; h-side multiplied from PSUM fp32 on DVE ('a') or fully evacuated and
  multiplied on DVE 2x ('b') / Pool ('c') per PH4_MIX; bf16 tree-sum.
- ph7a: project the 24 entity embeddings (not the 552 gathered pairs), then
  gather per pair with a K=24 matmul inside the ph7b PSUM accumulation.
- ph8: zh/zt replication tiles for the bilinear are materialized in SBUF
  bf16 via a DRAM round-trip (SBUF->DRAM reorder write, then DRAM->SBUF
  broadcast read with a step-0 mid-dim); SBUF-side partition-crossing DMAs
  are unreliable on HW, DRAM-side flat APs are exact.
- ph9: bilinear chunk layout (8 i's x 16 j's per 128-partition K-chunk):
  blt[p] = zh[g*64+bi*8+p//16] * zt[g*64+bj*16+p%16]; multiplies are
  all-bf16 DVE 2x_1p wide ops ([128, 4, 276], one per (g, bi), step-0
  broadcast on the rep operand), ~1/5 on Pool; accumulate = 384 K=128
  matmuls into one PSUM bank against host-permuted Wb chunks.
- Projections+ph8+ph9 run in two oc-halves so the second half's matmuls
  and DMAs overlap the first half's bilinear.
"""

import dataclasses
import numpy as np
import ml_dtypes

import concourse.bass as bass
import concourse.bacc as bacc
import concourse.tile as tile
from concourse import mybir
from concourse.bass_utils import run_bass_kernel_spmd

B, L, H, NH, E, M, P, NL, BLK = 4, 1024, 768, 12, 24, 4, 552, 97, 64
G = H // BLK            # 12 groups
R = P // 2              # 276 rows per core
EM = E * M              # 96 gathered mentions
HL = NH * L             # 12288
F32 = mybir.dt.float32
BF16 = mybir.dt.bfloat16
BF = ml_dtypes.bfloat16

# ph4 per-q engine mix for the 6 head-pair rounds:
#   'a' = t-evac only, DVE multiplies from PSUM fp32 (ACT 0.6us, DVE 1.0us)
#   'b' = full evac, DVE bf16 2x multiply     (ACT 1.2us, DVE 0.35us)
#   'c' = full evac, Pool bf16 multiply       (ACT 1.2us, Pool 1.2us)
PH4_MIX = ['a', 'a', 'a', 'a', 'c', 'c']
# ph9: units u with u % MOD == MOD-1 multiply on Pool, rest on DVE.
PH9_POOL_MOD = 5        # 19 of 96 units -> Pool


def _bcast(ap, n):
    """Insert a step-0 dim after the partition dim: [p, F] -> [p, n, F]."""
    return dataclasses.replace(ap, ap=[ap.ap[0], [0, n]] + ap.ap[1:])


def _build_program():
    nc = bacc.Bacc("TRN2", target_bir_lowering=False, debug=False, num_devices=8)
    dp = nc.declare_dram_parameter
    EMG = dp("EMG", [EM, H], F32, isOutput=False)       # gathered mention embeds
    SUME = dp("SUME", [EM, E], BF16, isOutput=False)    # mask one-hot
    AMG = dp("AMG", [EM, HL], BF16, isOutput=False)     # gathered attn rows (h-major)
    WHH = dp("WHH", [EM, R], BF16, isOutput=False)      # mask/denom/sqrtNH x head one-hot
    WTT = dp("WTT", [EM, R], BF16, isOutput=False)
    OHH = dp("OHH", [E, R], BF16, isOutput=False)       # head-entity one-hot
    OHT = dp("OHT", [E, R], BF16, isOutput=False)
    SEQ = dp("SEQ", [L, H], BF16, isOutput=False)
    W1H = dp("W1H", [H, H], BF16, isOutput=False)       # Wh[:, :768].T
    W2H = dp("W2H", [H, H], BF16, isOutput=False)       # Wh[:, 768:].T
    W1T = dp("W1T", [H, H], BF16, isOutput=False)       # Wt[perm, :768].T
    W2T = dp("W2T", [H, H], BF16, isOutput=False)
    WBT = dp("WBT", [128, 12, 32, NL], BF16, isOutput=False)  # permuted Wb.T
    BHS = dp("BHS", [128, 6], F32, isOutput=False)
    BTS = dp("BTS", [128, 6], F32, isOutput=False)
    BBS = dp("BBS", [NL, 1], F32, isOutput=False)
    XD = dp("XD", [8, G, 8, R], BF16, isOutput=False)   # scratch: zh reorder
    ZD = dp("ZD", [16, G, 4, R], BF16, isOutput=False)  # scratch: zt reorder
    OUT = dp("OUT", [NL, R], F32, isOutput=True)        # logits^T

    XDs, ZDs = 26496, 13248  # per-i / per-j row sizes (G*8*R, G*4*R)

    with tile.TileContext(nc) as tc:
        with (
            tc.tile_pool(name="persist", bufs=1) as pp,
            tc.tile_pool(name="wbt", bufs=8) as wbp,
            tc.tile_pool(name="blt", bufs=10) as bltp,
        ):
            # ---- persistent small loads (sync queue, program order = priority)
            def load(name_ap, shape, tag, dt=F32, eng=nc.sync):
                t = pp.tile(shape, dt, tag=tag, name=tag)
                eng.dma_start(t[:], name_ap)
                return t

            emg = load(EMG[:], [EM, H], "emg")
            sume = load(SUME[:], [EM, E], "sume", BF16)
            # amgp opened before w1p (LIFO pool closing: w1p closes first)
            amgp_cm = tc.tile_pool(name="amgp", bufs=1)
            amgp = amgp_cm.__enter__()
            w1p_cm = tc.tile_pool(name="w1p", bufs=1)
            w1p = w1p_cm.__enter__()
            whh = load(WHH[:], [EM, R], "whh", BF16)
            wtt = load(WTT[:], [EM, R], "wtt", BF16)
            ohh = load(OHH[:], [E, R], "ohh", BF16)
            oht = load(OHT[:], [E, R], "oht", BF16)
            bhs = load(BHS[:], [128, 6], "bhs")
            bts = load(BTS[:], [128, 6], "bts")
            bbs = load(BBS[:], [NL, 1], "bbs")
            # attention rows, 4 heads per load (pool scoped to ph4)
            amgq = []
            for hq in range(3):
                t = amgp.tile([EM, 4, L], BF16, tag=f"amg{hq}", name=f"amg{hq}")
                nc.sync.dma_start(
                    t[:], AMG[:, hq * 4 * L:(hq + 1) * 4 * L].rearrange(
                        "m (h l) -> m h l", h=4))
                amgq.append(t)
            amgt = [amgq[hp // 2][:, (hp % 2) * 2:(hp % 2) * 2 + 2, :]
                    for hp in range(6)]

            def load_rows_at(dram, off, n, pool, tag):
                t = pool.tile([128, n, H], BF16, tag=tag, name=tag)
                base = dram[:]
                src = dataclasses.replace(
                    base, offset=base.offset + off * 128 * H,
                    ap=[[H, 128], [128 * H, n], [1, H]])
                nc.sync.dma_start(t[:], src)
                return t

            # W1 weights next: ph7a runs right after ph4
            w1tp = [load_rows_at(W1T, 3 * i, 3, w1p, f"w1t{i}") for i in range(2)]
            w1hp = [load_rows_at(W1H, 3 * i, 3, w1p, f"w1h{i}") for i in range(2)]
            w1tt = [w1tp[dk // 3][:, dk % 3, :] for dk in range(6)]
            w1ht = [w1hp[dk // 3][:, dk % 3, :] for dk in range(6)]



            seqp = [load_rows_at(SEQ, 4 * i, 4, pp, f"seq{i}") for i in range(2)]
            seqt = [seqp[q // 4][:, q % 4, :] for q in range(8)]
            w2tp = [load_rows_at(W2T, 3 * i, 3, pp, f"w2t{i}") for i in range(2)]
            w2hp = [load_rows_at(W2H, 3 * i, 3, pp, f"w2h{i}") for i in range(2)]
            w2tt = [w2tp[dk // 3][:, dk % 3, :] for dk in range(6)]
            w2ht = [w2hp[dk // 3][:, dk % 3, :] for dk in range(6)]
            onesb = pp.tile([128, 1], BF16, tag="onesb", name="onesb")
            nc.vector.memset(onesb[:], 1.0)
            ones1 = pp.tile([1, 128], F32, tag="ones1", name="ones1")
            nc.vector.memset(ones1[:], 1.0)

            # 8 WBT loads prefetched on sync after all early-phase loads:
            # their transfers fill the DMA-idle ph4 window.
            wbts = []
            for i in range(8):
                t = wbp.tile([128, 32, NL], BF16, tag="wbc", name=f"wbc{i}")
                nc.sync.dma_start(t[:], WBT[:, i, :, :])
                wbts.append(t)

            # ---- ph1: eetT[d, e] = ln(sum_m sume[m, e] * exp(emg[m, d]))
            expt = pp.tile([EM, H], BF16, tag="expt", name="expt")
            nc.scalar.activation(expt[:], emg[:], mybir.ActivationFunctionType.Exp)
            eetT = []
            with tc.tile_pool(name="ps1", bufs=2, space="PSUM") as ps1:
                for dc in range(6):
                    pe = ps1.tile([128, E], F32, tag="ee_ps", name="ee_ps")
                    nc.tensor.matmul(pe[:], expt[:, dc * 128:(dc + 1) * 128],
                                     sume[:], start=True, stop=True)
                    t = pp.tile([128, E], BF16, tag=f"eetT{dc}", name=f"eetT{dc}")
                    nc.scalar.activation(t[:], pe[:], mybir.ActivationFunctionType.Ln)
                    eetT.append(t)

            # ---- ph4: ht_att accumulation per l-chunk q
            # hA/tA gathers K=96 (ph2 folded into WHH/WTT), 2 heads per round.
            htacc = []
            evp_cm = tc.tile_pool(name="evac", bufs=6)
            evp = evp_cm.__enter__()
            with (
                tc.tile_pool(name="ps4", bufs=2, space="PSUM") as ps4,
                tc.tile_pool(name="w4p", bufs=2) as w4p,
            ):
                def emit_tree(w4, q):
                    acc = pp.tile([128, R], BF16, tag=f"ht{q}", name=f"ht{q}")
                    nc.vector.tensor_add(w4[:, 0:6, :], w4[:, 0:6, :],
                                         w4[:, 6:12, :])
                    nc.gpsimd.tensor_add(w4[:, 0:3, :], w4[:, 0:3, :],
                                         w4[:, 3:6, :])
                    nc.vector.tensor_add(w4[:, 0, :], w4[:, 0, :], w4[:, 1, :])
                    nc.vector.tensor_add(acc[:], w4[:, 0, :], w4[:, 2, :])
                    htacc.append(acc)

                pending = None
                for q in range(8):
                    w4 = w4p.tile([128, NH, R], BF16, tag="w4", name="w4")
                    for hp in range(6):
                        if hp == 2 and pending is not None:
                            emit_tree(*pending)
                            pending = None
                        hh2 = ps4.tile([128, 2, 512], F32, tag="hh_ps", name="hh_ps")
                        tt2 = ps4.tile([128, 2, 512], F32, tag="tt_ps", name="tt_ps")
                        for kk in range(2):
                            amg_c = amgt[hp][:, kk, q * 128:(q + 1) * 128]
                            nc.tensor.matmul(hh2[:, kk, 0:R], amg_c, whh[:],
                                             start=True, stop=True)
                            nc.tensor.matmul(tt2[:, kk, 0:R], amg_c, wtt[:],
                                             start=True, stop=True)
                        # evacuate t-side (ACT), multiply per PH4_MIX
                        mode = PH4_MIX[hp]
                        tsb = evp.tile([128, 2, R], BF16, tag="tsb", name="tsb")
                        nc.scalar.copy(tsb[:], tt2[:, :, 0:R])
                        if mode == 'a':
                            nc.vector.tensor_tensor(
                                w4[:, hp * 2:hp * 2 + 2, :], hh2[:, :, 0:R],
                                tsb[:], mybir.AluOpType.mult)
                        else:
                            hsb = evp.tile([128, 2, R], BF16, tag="hsb", name="hsb")
                            nc.scalar.copy(hsb[:], hh2[:, :, 0:R])
                            eng = nc.vector if mode == 'b' else nc.gpsimd
                            eng.tensor_tensor(
                                w4[:, hp * 2:hp * 2 + 2, :], hsb[:], tsb[:],
                                mybir.AluOpType.mult)
                    # tree deferred into the next q's rounds (keeps the
                    # DVE queue from stalling on the Pool hop)
                    pending = (w4, q)
                emit_tree(*pending)
            evp_cm.__exit__(None, None, None)

            # ---- ph7a: entity-side projections ph_eT[e, o] (h and t)
            pheT, pteT = [], []
            for (w1l, outl, tag) in ((w1tt, pteT, "pte"), (w1ht, pheT, "phe")):
                with tc.tile_pool(name=f"ps7a{tag}", bufs=1, space="PSUM") as ps7a:
                    pgs = [ps7a.tile([E, 128], F32, tag=f"{tag}ps{oc}",
                                     name=f"{tag}ps{oc}") for oc in range(6)]
                    for dk in range(6):
                        for oc in range(6):
                            nc.tensor.matmul(pgs[oc][:], eetT[dk][:],
                                             w1l[dk][:, oc * 128:(oc + 1) * 128],
                                             start=(dk == 0), stop=(dk == 5))
                    for oc in range(6):
                        t = pp.tile([E, 128], BF16, tag=f"{tag}{oc}", name=f"{tag}{oc}")
                        nc.scalar.copy(t[:], pgs[oc][:])
                        outl.append(t)
            w1p_cm.__exit__(None, None, None)

            w1p_cm.__exit__(None, None, None)
            amgp_cm.__exit__(None, None, None)

            # ---- ph5: invd = 1/(sum_l ht + 1e-5), broadcast to 128 partitions
            invd = pp.tile([128, R], F32, tag="invd", name="invd")
            with tc.tile_pool(name="ps5", bufs=1, space="PSUM") as ps5:
                psum_s = ps5.tile([1, R], F32, tag="s_ps", name="s_ps")
                for q in range(8):
                    nc.tensor.matmul(psum_s[:], onesb[:], htacc[q][:],
                                     start=(q == 0), stop=(q == 7))
                invd1 = pp.tile([1, R], F32, tag="invd1", name="invd1")
                nc.vector.tensor_scalar_add(invd1[:], psum_s[:], 1e-5)
                nc.vector.reciprocal(invd1[:], invd1[:])
                pb = ps5.tile([128, R], F32, tag="invd_ps", name="invd_ps")
                nc.tensor.matmul(pb[:], ones1[:], invd1[:], start=True, stop=True)
                nc.scalar.copy(invd[:], pb[:])


            # ---- ph6: rs^T chunks (normalization folded into evac)
            rst = []
            with tc.tile_pool(name="ps6", bufs=2, space="PSUM") as ps6:
                for dc in range(6):
                    pr = ps6.tile([128, R], F32, tag="rs_ps", name="rs_ps")
                    for q in range(8):
                        nc.tensor.matmul(pr[:], seqt[q][:, dc * 128:(dc + 1) * 128],
                                         htacc[q][:], start=(q == 0), stop=(q == 7))
                    t = pp.tile([128, R], BF16, tag=f"rs{dc}", name=f"rs{dc}")
                    nc.vector.tensor_mul(t[:], pr[:], invd[:])
                    rst.append(t)

            # ---- ph7b + ph8 + ph9, split in two oc-halves so the second
            # half's projections and DMAs overlap the first half's bilinear.
            repp_cms = [tc.tile_pool(name=f"repp{i}", bufs=1) for i in range(12)]
            repps = [cm.__enter__() for cm in repp_cms]
            rept2 = [repps[gp].tile([128, 2, 8, R], BF16, tag=f"rep{gp}",
                                    name=f"rep{gp}") for gp in range(6)]
            ztr2 = [repps[6 + gp].tile([128, 2, 4, R], BF16, tag=f"ztr{gp}",
                                       name=f"ztr{gp}") for gp in range(6)]
            zht, ztt = [[None] * 6 for _ in range(2)]

            def project_half(half):
                # zt and zh pools open together; per-oc alternation gets the
                # zh-side rep reads onto the DMA queue as early as possible
                ocs = range(half * 3, half * 3 + 3)
                pt_cm = tc.tile_pool(name=f"ps7zt{half}", bufs=1, space="PSUM")
                pt = pt_cm.__enter__()
                ph_cm = tc.tile_pool(name=f"ps7zh{half}", bufs=1, space="PSUM")
                ph_ = ph_cm.__enter__()
                for oc in ocs:
                    for (pool, w2l, phe, oh, bias, outl, tag) in (
                            (ph_, w2ht, pheT, ohh, bhs, zht, "zh"),
                            (pt, w2tt, pteT, oht, bts, ztt, "zt")):
                        pps = pool.tile([128, R], F32, tag=f"{tag}ps{oc}",
                                        name=f"{tag}ps{oc}")
                        nc.tensor.matmul(pps[:], phe[oc][:], oh[:],
                                         start=True, stop=False)
                        for dk in range(6):
                            nc.tensor.matmul(
                                pps[:], w2l[dk][:, oc * 128:(oc + 1) * 128],
                                rst[dk][:], start=False, stop=(dk == 5))
                        t = pp.tile([128, R], BF16, tag=f"{tag}{oc}",
                                    name=f"{tag}{oc}")
                        nc.scalar.activation(
                            t[:], pps[:], mybir.ActivationFunctionType.Tanh,
                            bias=bias[:, oc:oc + 1])
                        outl[oc] = t
                        if tag == "zt":
                            for gh in range(2):
                                g = oc * 2 + gh
                                dst = dataclasses.replace(
                                    ZD[:], offset=ZD[:].offset + g * 4 * R,
                                    ap=[[ZDs, 16], [R, 4], [1, R]])
                                nc.sync.dma_start(dst, t[gh * 64:(gh + 1) * 64, :])
                            srcz = dataclasses.replace(
                                ZD[:], offset=ZD[:].offset + oc * 2 * 4 * R,
                                ap=[[0, 8], [ZDs, 16], [1, 2 * 4 * R]])
                            dstz = ztr2[oc][:].rearrange("p g bj r -> p (g bj r)")
                            nc.sync.dma_start(dstz, srcz)
                        else:
                            for gh in range(2):
                                g = oc * 2 + gh
                                dst = dataclasses.replace(
                                    XD[:], offset=XD[:].offset + g * 8 * R,
                                    ap=[[R, 8], [XDs, 8], [1, R]])
                                nc.sync.dma_start(dst, t[gh * 64:(gh + 1) * 64, :])
                            srcx = dataclasses.replace(
                                XD[:], offset=XD[:].offset + oc * 2 * 8 * R,
                                ap=[[XDs, 8], [0, 16], [1, 2 * 8 * R]])
                            dstx = rept2[oc][:].rearrange("p g bi r -> p (g bi r)")
                            nc.sync.dma_start(dstx, srcx)
                ph_cm.__exit__(None, None, None)
                pt_cm.__exit__(None, None, None)

            with (
                tc.tile_pool(name="ps9", bufs=1, space="PSUM") as ps9,
            ):
                lt = ps9.tile([NL, R], F32, tag="lt_ps", name="lt_ps")
                state = {"ci": 0, "u": 0}

                def ph9_groups(gs):
                    for g in gs:
                        for bi in range(8):
                            blt = bltp.tile([128, 4, R], BF16, tag="blt",
                                            name="blt")
                            rep_b = _bcast(rept2[g // 2][:, g % 2, bi, :], 4)
                            ztr_v = ztr2[g // 2][:, g % 2, :, :]
                            if state["u"] % PH9_POOL_MOD == 2:
                                nc.gpsimd.tensor_tensor(
                                    blt[:], rep_b, ztr_v, mybir.AluOpType.mult)
                            else:
                                nc.vector.tensor_tensor(
                                    blt[:], rep_b, ztr_v, mybir.AluOpType.mult)
                            state["u"] += 1
                            for bj in range(4):
                                ci = state["ci"]
                                nc.tensor.matmul(
                                    lt[:], wbts[ci // 32][:, ci % 32, :],
                                    blt[:, bj, :],
                                    start=(ci == 0), stop=(ci == 383))
                                state["ci"] += 1

                project_half(0)
                ph9_groups(range(0, 4))
                project_half(1)
                for i in range(8, 12):
                    t = wbp.tile([128, 32, NL], BF16, tag="wbc", name=f"wbc{i}")
                    nc.sync.dma_start(t[:], WBT[:, i, :, :])
                    wbts.append(t)
                ph9_groups(range(4, 12))

                lout = pp.tile([NL, R], F32, tag="lout", name="lout")
                nc.vector.tensor_scalar_add(lout[:], lt[:], bbs[:, 0:1])
                nc.sync.dma_start(OUT[:], lout[:])
            for cm in reversed(repp_cms):
                cm.__exit__(None, None, None)

    nc.finalize()
    return nc


def _t_perm():
    """Store-row permutation for the zt side: store q=g*64+j*4+bj holds
    logical o=g*64+bj*16+j."""
    perm = np.empty(H, np.int64)
    for g in range(G):
        for j in range(16):
            for bj in range(4):
                perm[g * 64 + j * 4 + bj] = g * 64 + bj * 16 + j
    return perm


def _wbt_perm(Wb):
    """WBT[p, load, slot, n] = Wb.T[k, n] for chunk ci=load*16+slot,
    k = g*4096 + i*64 + j, i = bi*8 + p//16, j = bj*16 + p%16,
    ci = g*32 + bi*4 + bj."""
    wbt = Wb.T  # [K, NL]
    p = np.arange(128)
    ip, jp = p // 16, p % 16
    rows = np.empty((384, 128), np.int64)
    for g in range(G):
        for bi in range(8):
            for bj in range(4):
                ci = g * 32 + bi * 4 + bj
                rows[ci] = g * 4096 + (bi * 8 + ip) * 64 + (bj * 16 + jp)
    w = wbt[rows]                                # [384, 128, NL]
    return np.ascontiguousarray(
        w.reshape(12, 32, 128, NL).transpose(2, 0, 1, 3)).astype(BF)


_CACHE = {}


def _prep_core_inputs(c, sequence_output, attention, mention_mask, Wh, bh, Wt, bt,
                      Wb, bb, mention_idx, hts):
    b, half = c // 2, c % 2
    seq_b = np.ascontiguousarray(sequence_output[b])              # [L, H]
    idx = mention_idx[b].astype(np.int64).reshape(EM)             # [96]
    mask = mention_mask[b].astype(np.float32)                     # [E, M]
    denom = mask.sum(-1)                                          # [E]

    emg = np.ascontiguousarray(seq_b[idx])                        # [96, H]
    amg = np.ascontiguousarray(
        attention[b][:, idx, :].transpose(1, 0, 2).reshape(EM, HL))

    sume = np.zeros((EM, E), np.float32)
    for e in range(E):
        for m in range(M):
            sume[e * M + m, e] = mask[e, m]

    hts_c = hts[b, half * R:(half + 1) * R].astype(np.int64)      # [R, 2]
    s = 1.0 / np.sqrt(np.float32(NH))
    wm = (mask / denom[:, None] * s).reshape(EM)                  # [96]
    whh = wm[:, None] * (hts_c[None, :, 0] == (np.arange(EM) // M)[:, None])
    wtt = wm[:, None] * (hts_c[None, :, 1] == (np.arange(EM) // M)[:, None])
    ohh = (hts_c[None, :, 0] == np.arange(E)[:, None]).astype(np.float32)
    oht = (hts_c[None, :, 1] == np.arange(E)[:, None]).astype(np.float32)

    tp = _CACHE.setdefault("tperm", _t_perm())
    w1h = np.ascontiguousarray(Wh[:, :H].T)                       # [768, 768]
    w2h = np.ascontiguousarray(Wh[:, H:].T)
    w1t = np.ascontiguousarray(Wt[tp, :H].T)
    w2t = np.ascontiguousarray(Wt[tp, H:].T)

    if "wbt" not in _CACHE:
        _CACHE["wbt"] = _wbt_perm(Wb)

    return {
        "EMG": emg.astype(np.float32),
        "SUME": sume.astype(BF),
        "AMG": amg.astype(BF),
        "WHH": whh.astype(BF), "WTT": wtt.astype(BF),
        "OHH": ohh.astype(BF), "OHT": oht.astype(BF),
        "SEQ": seq_b.astype(BF),
        "W1H": w1h.astype(BF), "W2H": w2h.astype(BF),
        "W1T": w1t.astype(BF), "W2T": w2t.astype(BF),
        "WBT": _CACHE["wbt"],
        "BHS": np.ascontiguousarray(bh.reshape(6, 128).T).astype(np.float32),
        "BTS": np.ascontiguousarray(bt[tp].reshape(6, 128).T).astype(np.float32),
        "BBS": bb.reshape(NL, 1).astype(np.float32),
        "XD": np.zeros((8, G, 8, R), BF),
        "ZD": np.zeros((16, G, 4, R), BF),
    }


def kernel(sequence_output, attention, mention_mask, Wh, bh, Wt, bt, Wb, bb,
           mention_idx, hts):
    if "nc" not in _CACHE:
        _CACHE["nc"] = _build_program()
    nc = _CACHE["nc"]

    args = (np.asarray(sequence_output, np.float32), np.asarray(attention, np.float32),
            np.asarray(mention_mask, np.float32), np.asarray(Wh, np.float32),
            np.asarray(bh, np.float32), np.asarray(Wt, np.float32),
            np.asarray(bt, np.float32), np.asarray(Wb, np.float32),
            np.asarray(bb, np.float32), np.asarray(mention_idx),
            np.asarray(hts))
    _CACHE.pop("wbt", None)   # Wb may differ between calls
    in_maps = [_prep_core_inputs(c, *args) for c in range(8)]
    res = None
    for attempt in range(4):
        try:
            res = run_bass_kernel_spmd(nc, in_maps, list(range(8))).results
            break
        except Exception:
            # transient NRT_EXEC_UNIT_UNRECOVERABLE is occasionally raised by
            # the runtime; back off briefly and retry
            if attempt == 3:
                raise
            import time
            time.sleep(2.0 * (attempt + 1))

    out = np.empty((B, P, NL), np.float32)
    for c in range(8):
        b, half = c // 2, c % 2
        out[b, half * R:(half + 1) * R, :] = np.asarray(res[c]["OUT"]).T
    return out


# revision 46
# speedup vs baseline: 1.7182x; 1.0313x over previous
"""Bass/Trainium2 kernel for DocRE bilinear segment-reduce model (v2).

Shapes (hardcoded): B=4, L=1024, H=768, NH=12, E=24, M=4, P=552, NL=97, BLK=64.
Sharding: 8 cores = (batch b = core//2) x (half of the 552 head-tail pairs).
Host prep is index-only (gathers/one-hots/permutations); all value compute
runs on device. TimelineSim-guided design; key structure:

- ph1: entity embeddings produced directly transposed (eetT [128d, 24e] x 6).
- ph2 folded into ph4: host combines the mention-mask weights with the pair
  one-hots (WHH/WTT [96, R]) so the entity-attention gathers contract K=96
  in one matmul per (head, l-chunk); no separate entity-attention pass.
- ph4: per l-chunk q, 6 head-pair rounds; t-side evacuated to SBUF bf16 by
  ACT; h-side multiplied from PSUM fp32 on DVE ('a') or fully evacuated and
  multiplied on DVE 2x ('b') / Pool ('c') per PH4_MIX; bf16 tree-sum.
- ph7a: project the 24 entity embeddings (not the 552 gathered pairs), then
  gather per pair with a K=24 matmul inside the ph7b PSUM accumulation.
- ph8: zh/zt replication tiles for the bilinear are materialized in SBUF
  bf16 via a DRAM round-trip (SBUF->DRAM reorder write, then DRAM->SBUF
  broadcast read with a step-0 mid-dim); SBUF-side partition-crossing DMAs
  are unreliable on HW, DRAM-side flat APs are exact.
- ph9: bilinear chunk layout (8 i's x 16 j's per 128-partition K-chunk):
  blt[p] = zh[g*64+bi*8+p//16] * zt[g*64+bj*16+p%16]; multiplies are
  all-bf16 DVE 2x_1p wide ops ([128, 4, 276], one per (g, bi), step-0
  broadcast on the rep operand), ~1/5 on Pool; accumulate = 384 K=128
  matmuls into one PSUM bank against host-permuted Wb chunks.
- Projections+ph8+ph9 run in two oc-halves so the second half's matmuls
  and DMAs overlap the first half's bilinear.
"""

import dataclasses
import numpy as np
import ml_dtypes

import concourse.bass as bass
import concourse.bacc as bacc
import concourse.tile as tile
from concourse import mybir
from concourse.bass_utils import run_bass_kernel_spmd

B, L, H, NH, E, M, P, NL, BLK = 4, 1024, 768, 12, 24, 4, 552, 97, 64
G = H // BLK            # 12 groups
R = P // 2              # 276 rows per core
EM = E * M              # 96 gathered mentions
HL = NH * L             # 12288
F32 = mybir.dt.float32
BF16 = mybir.dt.bfloat16
BF = ml_dtypes.bfloat16

# ph4 per-q engine mix for the 6 head-pair rounds:
#   'a' = t-evac only, DVE multiplies from PSUM fp32 (ACT 0.6us, DVE 1.0us)
#   'b' = full evac, DVE bf16 2x multiply     (ACT 1.2us, DVE 0.35us)
#   'c' = full evac, Pool bf16 multiply       (ACT 1.2us, Pool 1.2us)
PH4_MIX = ['c','a','a','c','a','a']
# ph9: units u with u % MOD == MOD-1 multiply on Pool, rest on DVE.
PH9_POOL_MOD = 5        # 19 of 96 units -> Pool


def _bcast(ap, n):
    """Insert a step-0 dim after the partition dim: [p, F] -> [p, n, F]."""
    return dataclasses.replace(ap, ap=[ap.ap[0], [0, n]] + ap.ap[1:])


def _build_program():
    nc = bacc.Bacc("TRN2", target_bir_lowering=False, debug=False, num_devices=8)
    dp = nc.declare_dram_parameter
    EMG = dp("EMG", [EM, H], F32, isOutput=False)       # gathered mention embeds
    SUME = dp("SUME", [EM, E], BF16, isOutput=False)    # mask one-hot
    AMG = dp("AMG", [EM, HL], BF16, isOutput=False)     # gathered attn rows (h-major)
    WHH = dp("WHH", [EM, R], BF16, isOutput=False)      # mask/denom/sqrtNH x head one-hot
    WTT = dp("WTT", [EM, R], BF16, isOutput=False)
    OHH = dp("OHH", [E, R], BF16, isOutput=False)       # head-entity one-hot
    OHT = dp("OHT", [E, R], BF16, isOutput=False)
    SEQ = dp("SEQ", [L, H], BF16, isOutput=False)
    W1H = dp("W1H", [H, H], BF16, isOutput=False)       # Wh[:, :768].T
    W2H = dp("W2H", [H, H], BF16, isOutput=False)       # Wh[:, 768:].T
    W1T = dp("W1T", [H, H], BF16, isOutput=False)       # Wt[perm, :768].T
    W2T = dp("W2T", [H, H], BF16, isOutput=False)
    WBT = dp("WBT", [128, 12, 32, NL], BF16, isOutput=False)  # permuted Wb.T
    BHS = dp("BHS", [128, 6], F32, isOutput=False)
    BTS = dp("BTS", [128, 6], F32, isOutput=False)
    BBS = dp("BBS", [NL, 1], F32, isOutput=False)
    XD = dp("XD", [8, G, 8, R], BF16, isOutput=False)   # scratch: zh reorder
    ZD = dp("ZD", [16, G, 4, R], BF16, isOutput=False)  # scratch: zt reorder
    OUT = dp("OUT", [NL, R], F32, isOutput=True)        # logits^T

    XDs, ZDs = 26496, 13248  # per-i / per-j row sizes (G*8*R, G*4*R)

    with tile.TileContext(nc) as tc:
        with (
            tc.tile_pool(name="persist", bufs=1) as pp,
            tc.tile_pool(name="wbt", bufs=8) as wbp,
            tc.tile_pool(name="blt", bufs=10) as bltp,
        ):
            # ---- persistent small loads (sync queue, program order = priority)
            def load(name_ap, shape, tag, dt=F32, eng=nc.sync):
                t = pp.tile(shape, dt, tag=tag, name=tag)
                eng.dma_start(t[:], name_ap)
                return t

            emg = load(EMG[:], [EM, H], "emg")
            sume = load(SUME[:], [EM, E], "sume", BF16)
            # amgp opened before w1p (LIFO pool closing: w1p closes first)
            amgp_cm = tc.tile_pool(name="amgp", bufs=1)
            amgp = amgp_cm.__enter__()
            w1p_cm = tc.tile_pool(name="w1p", bufs=1)
            w1p = w1p_cm.__enter__()
            whh = load(WHH[:], [EM, R], "whh", BF16)
            wtt = load(WTT[:], [EM, R], "wtt", BF16)
            ohh = load(OHH[:], [E, R], "ohh", BF16)
            oht = load(OHT[:], [E, R], "oht", BF16)
            bhs = load(BHS[:], [128, 6], "bhs")
            bts = load(BTS[:], [128, 6], "bts")
            bbs = load(BBS[:], [NL, 1], "bbs")
            # attention rows, 4 heads per load (pool scoped to ph4)
            amgq = []
            for hq in range(3):
                t = amgp.tile([EM, 4, L], BF16, tag=f"amg{hq}", name=f"amg{hq}")
                nc.sync.dma_start(
                    t[:], AMG[:, hq * 4 * L:(hq + 1) * 4 * L].rearrange(
                        "m (h l) -> m h l", h=4))
                amgq.append(t)
            amgt = [amgq[hp // 2][:, (hp % 2) * 2:(hp % 2) * 2 + 2, :]
                    for hp in range(6)]

            def load_rows_at(dram, off, n, pool, tag):
                t = pool.tile([128, n, H], BF16, tag=tag, name=tag)
                base = dram[:]
                src = dataclasses.replace(
                    base, offset=base.offset + off * 128 * H,
                    ap=[[H, 128], [128 * H, n], [1, H]])
                nc.sync.dma_start(t[:], src)
                return t

            # W1 weights next: ph7a runs right after ph4
            w1tp = [load_rows_at(W1T, 3 * i, 3, w1p, f"w1t{i}") for i in range(2)]
            w1hp = [load_rows_at(W1H, 3 * i, 3, w1p, f"w1h{i}") for i in range(2)]
            w1tt = [w1tp[dk // 3][:, dk % 3, :] for dk in range(6)]
            w1ht = [w1hp[dk // 3][:, dk % 3, :] for dk in range(6)]



            seqp = [load_rows_at(SEQ, 4 * i, 4, pp, f"seq{i}") for i in range(2)]
            seqt = [seqp[q // 4][:, q % 4, :] for q in range(8)]
            w2tp = [load_rows_at(W2T, 3 * i, 3, pp, f"w2t{i}") for i in range(2)]
            w2hp = [load_rows_at(W2H, 3 * i, 3, pp, f"w2h{i}") for i in range(2)]
            w2tt = [w2tp[dk // 3][:, dk % 3, :] for dk in range(6)]
            w2ht = [w2hp[dk // 3][:, dk % 3, :] for dk in range(6)]
            onesb = pp.tile([128, 1], BF16, tag="onesb", name="onesb")
            nc.vector.memset(onesb[:], 1.0)
            ones1 = pp.tile([1, 128], F32, tag="ones1", name="ones1")
            nc.vector.memset(ones1[:], 1.0)

            # 8 WBT loads prefetched on sync after all early-phase loads:
            # their transfers fill the DMA-idle ph4 window.
            wbts = []
            for i in range(8):
                t = wbp.tile([128, 32, NL], BF16, tag="wbc", name=f"wbc{i}")
                nc.sync.dma_start(t[:], WBT[:, i, :, :])
                wbts.append(t)

            # ---- ph1: eetT[d, e] = ln(sum_m sume[m, e] * exp(emg[m, d]))
            expt = pp.tile([EM, H], BF16, tag="expt", name="expt")
            nc.scalar.activation(expt[:], emg[:], mybir.ActivationFunctionType.Exp)
            eetT = []
            with tc.tile_pool(name="ps1", bufs=2, space="PSUM") as ps1:
                for dc in range(6):
                    pe = ps1.tile([128, E], F32, tag="ee_ps", name="ee_ps")
                    nc.tensor.matmul(pe[:], expt[:, dc * 128:(dc + 1) * 128],
                                     sume[:], start=True, stop=True)
                    t = pp.tile([128, E], BF16, tag=f"eetT{dc}", name=f"eetT{dc}")
                    nc.scalar.activation(t[:], pe[:], mybir.ActivationFunctionType.Ln)
                    eetT.append(t)

            # ---- ph4: ht_att accumulation per l-chunk q
            # hA/tA gathers K=96 (ph2 folded into WHH/WTT), 2 heads per round.
            htacc = []
            evp_cm = tc.tile_pool(name="evac", bufs=6)
            evp = evp_cm.__enter__()
            with (
                tc.tile_pool(name="ps4", bufs=2, space="PSUM") as ps4,
                tc.tile_pool(name="w4p", bufs=2) as w4p,
            ):
                def emit_tree(w4, q):
                    acc = pp.tile([128, R], BF16, tag=f"ht{q}", name=f"ht{q}")
                    nc.vector.tensor_add(w4[:, 0:6, :], w4[:, 0:6, :],
                                         w4[:, 6:12, :])
                    nc.gpsimd.tensor_add(w4[:, 0:3, :], w4[:, 0:3, :],
                                         w4[:, 3:6, :])
                    nc.vector.tensor_add(w4[:, 0, :], w4[:, 0, :], w4[:, 1, :])
                    nc.vector.tensor_add(acc[:], w4[:, 0, :], w4[:, 2, :])
                    htacc.append(acc)

                pending = None
                for q in range(8):
                    w4 = w4p.tile([128, NH, R], BF16, tag="w4", name="w4")
                    for hp in range(6):
                        if hp == 2 and pending is not None:
                            emit_tree(*pending)
                            pending = None
                        hh2 = ps4.tile([128, 2, 512], F32, tag="hh_ps", name="hh_ps")
                        tt2 = ps4.tile([128, 2, 512], F32, tag="tt_ps", name="tt_ps")
                        for kk in range(2):
                            amg_c = amgt[hp][:, kk, q * 128:(q + 1) * 128]
                            nc.tensor.matmul(hh2[:, kk, 0:R], amg_c, whh[:],
                                             start=True, stop=True)
                            nc.tensor.matmul(tt2[:, kk, 0:R], amg_c, wtt[:],
                                             start=True, stop=True)
                        # evacuate t-side (ACT), multiply per PH4_MIX
                        mode = PH4_MIX[hp]
                        tsb = evp.tile([128, 2, R], BF16, tag="tsb", name="tsb")
                        nc.scalar.copy(tsb[:], tt2[:, :, 0:R])
                        if mode == 'a':
                            nc.vector.tensor_tensor(
                                w4[:, hp * 2:hp * 2 + 2, :], hh2[:, :, 0:R],
                                tsb[:], mybir.AluOpType.mult)
                        else:
                            hsb = evp.tile([128, 2, R], BF16, tag="hsb", name="hsb")
                            nc.scalar.copy(hsb[:], hh2[:, :, 0:R])
                            eng = nc.vector if mode == 'b' else nc.gpsimd
                            eng.tensor_tensor(
                                w4[:, hp * 2:hp * 2 + 2, :], hsb[:], tsb[:],
                                mybir.AluOpType.mult)
                    # tree deferred into the next q's rounds (keeps the
                    # DVE queue from stalling on the Pool hop)
                    pending = (w4, q)
                emit_tree(*pending)
            evp_cm.__exit__(None, None, None)

            # ---- ph7a: entity-side projections ph_eT[e, o] (h and t)
            pheT, pteT = [], []
            for (w1l, outl, tag) in ((w1tt, pteT, "pte"), (w1ht, pheT, "phe")):
                with tc.tile_pool(name=f"ps7a{tag}", bufs=1, space="PSUM") as ps7a:
                    pgs = [ps7a.tile([E, 128], F32, tag=f"{tag}ps{oc}",
                                     name=f"{tag}ps{oc}") for oc in range(6)]
                    for dk in range(6):
                        for oc in range(6):
                            nc.tensor.matmul(pgs[oc][:], eetT[dk][:],
                                             w1l[dk][:, oc * 128:(oc + 1) * 128],
                                             start=(dk == 0), stop=(dk == 5))
                    for oc in range(6):
                        t = pp.tile([E, 128], BF16, tag=f"{tag}{oc}", name=f"{tag}{oc}")
                        nc.scalar.copy(t[:], pgs[oc][:])
                        outl.append(t)
            w1p_cm.__exit__(None, None, None)

            w1p_cm.__exit__(None, None, None)
            amgp_cm.__exit__(None, None, None)

            # ---- ph5: invd = 1/(sum_l ht + 1e-5), broadcast to 128 partitions
            invd = pp.tile([128, R], F32, tag="invd", name="invd")
            with tc.tile_pool(name="ps5", bufs=1, space="PSUM") as ps5:
                psum_s = ps5.tile([1, R], F32, tag="s_ps", name="s_ps")
                for q in range(8):
                    nc.tensor.matmul(psum_s[:], onesb[:], htacc[q][:],
                                     start=(q == 0), stop=(q == 7))
                invd1 = pp.tile([1, R], F32, tag="invd1", name="invd1")
                nc.vector.tensor_scalar_add(invd1[:], psum_s[:], 1e-5)
                nc.vector.reciprocal(invd1[:], invd1[:])
                pb = ps5.tile([128, R], F32, tag="invd_ps", name="invd_ps")
                nc.tensor.matmul(pb[:], ones1[:], invd1[:], start=True, stop=True)
                nc.scalar.copy(invd[:], pb[:])


            # ---- ph6: rs^T chunks (normalization folded into evac)
            rst = []
            with tc.tile_pool(name="ps6", bufs=2, space="PSUM") as ps6:
                for dc in range(6):
                    pr = ps6.tile([128, R], F32, tag="rs_ps", name="rs_ps")
                    for q in range(8):
                        nc.tensor.matmul(pr[:], seqt[q][:, dc * 128:(dc + 1) * 128],
                                         htacc[q][:], start=(q == 0), stop=(q == 7))
                    t = pp.tile([128, R], BF16, tag=f"rs{dc}", name=f"rs{dc}")
                    nc.vector.tensor_mul(t[:], pr[:], invd[:])
                    rst.append(t)

            # ---- ph7b + ph8 + ph9, split in two oc-halves so the second
            # half's projections and DMAs overlap the first half's bilinear.
            repp_cms = [tc.tile_pool(name=f"repp{i}", bufs=1) for i in range(12)]
            repps = [cm.__enter__() for cm in repp_cms]
            rept2 = [repps[gp].tile([128, 2, 8, R], BF16, tag=f"rep{gp}",
                                    name=f"rep{gp}") for gp in range(6)]
            ztr2 = [repps[6 + gp].tile([128, 2, 4, R], BF16, tag=f"ztr{gp}",
                                       name=f"ztr{gp}") for gp in range(6)]
            zht, ztt = [[None] * 6 for _ in range(2)]

            def project_half(half):
                # zt and zh pools open together; per-oc alternation gets the
                # zh-side rep reads onto the DMA queue as early as possible
                ocs = range(half * 3, half * 3 + 3)
                pt_cm = tc.tile_pool(name=f"ps7zt{half}", bufs=1, space="PSUM")
                pt = pt_cm.__enter__()
                ph_cm = tc.tile_pool(name=f"ps7zh{half}", bufs=1, space="PSUM")
                ph_ = ph_cm.__enter__()
                for oc in ocs:
                    for (pool, w2l, phe, oh, bias, outl, tag) in (
                            (ph_, w2ht, pheT, ohh, bhs, zht, "zh"),
                            (pt, w2tt, pteT, oht, bts, ztt, "zt")):
                        pps = pool.tile([128, R], F32, tag=f"{tag}ps{oc}",
                                        name=f"{tag}ps{oc}")
                        nc.tensor.matmul(pps[:], phe[oc][:], oh[:],
                                         start=True, stop=False)
                        for dk in range(6):
                            nc.tensor.matmul(
                                pps[:], w2l[dk][:, oc * 128:(oc + 1) * 128],
                                rst[dk][:], start=False, stop=(dk == 5))
                        t = pp.tile([128, R], BF16, tag=f"{tag}{oc}",
                                    name=f"{tag}{oc}")
                        nc.scalar.activation(
                            t[:], pps[:], mybir.ActivationFunctionType.Tanh,
                            bias=bias[:, oc:oc + 1])
                        outl[oc] = t
                        if tag == "zt":
                            for gh in range(2):
                                g = oc * 2 + gh
                                dst = dataclasses.replace(
                                    ZD[:], offset=ZD[:].offset + g * 4 * R,
                                    ap=[[ZDs, 16], [R, 4], [1, R]])
                                nc.sync.dma_start(dst, t[gh * 64:(gh + 1) * 64, :])
                            srcz = dataclasses.replace(
                                ZD[:], offset=ZD[:].offset + oc * 2 * 4 * R,
                                ap=[[0, 8], [ZDs, 16], [1, 2 * 4 * R]])
                            dstz = ztr2[oc][:].rearrange("p g bj r -> p (g bj r)")
                            nc.sync.dma_start(dstz, srcz)
                        else:
                            for gh in range(2):
                                g = oc * 2 + gh
                                dst = dataclasses.replace(
                                    XD[:], offset=XD[:].offset + g * 8 * R,
                                    ap=[[R, 8], [XDs, 8], [1, R]])
                                nc.sync.dma_start(dst, t[gh * 64:(gh + 1) * 64, :])
                            srcx = dataclasses.replace(
                                XD[:], offset=XD[:].offset + oc * 2 * 8 * R,
                                ap=[[XDs, 8], [0, 16], [1, 2 * 8 * R]])
                            dstx = rept2[oc][:].rearrange("p g bi r -> p (g bi r)")
                            nc.sync.dma_start(dstx, srcx)
                ph_cm.__exit__(None, None, None)
                pt_cm.__exit__(None, None, None)

            with (
                tc.tile_pool(name="ps9", bufs=1, space="PSUM") as ps9,
            ):
                lt = ps9.tile([NL, R], F32, tag="lt_ps", name="lt_ps")
                state = {"ci": 0, "u": 0}

                def ph9_groups(gs):
                    for g in gs:
                        for bi in range(8):
                            blt = bltp.tile([128, 4, R], BF16, tag="blt",
                                            name="blt")
                            rep_b = _bcast(rept2[g // 2][:, g % 2, bi, :], 4)
                            ztr_v = ztr2[g // 2][:, g % 2, :, :]
                            if state["u"] % PH9_POOL_MOD == 2:
                                nc.gpsimd.tensor_tensor(
                                    blt[:], rep_b, ztr_v, mybir.AluOpType.mult)
                            else:
                                nc.vector.tensor_tensor(
                                    blt[:], rep_b, ztr_v, mybir.AluOpType.mult)
                            state["u"] += 1
                            for bj in range(4):
                                ci = state["ci"]
                                nc.tensor.matmul(
                                    lt[:], wbts[ci // 32][:, ci % 32, :],
                                    blt[:, bj, :],
                                    start=(ci == 0), stop=(ci == 383))
                                state["ci"] += 1

                project_half(0)
                ph9_groups(range(0, 4))
                project_half(1)
                for i in range(8, 12):
                    t = wbp.tile([128, 32, NL], BF16, tag="wbc", name=f"wbc{i}")
                    nc.sync.dma_start(t[:], WBT[:, i, :, :])
                    wbts.append(t)
                ph9_groups(range(4, 12))

                lout = pp.tile([NL, R], F32, tag="lout", name="lout")
                nc.vector.tensor_scalar_add(lout[:], lt[:], bbs[:, 0:1])
                nc.sync.dma_start(OUT[:], lout[:])
            for cm in reversed(repp_cms):
                cm.__exit__(None, None, None)

    nc.finalize()
    return nc


def _t_perm():
    """Store-row permutation for the zt side: store q=g*64+j*4+bj holds
    logical o=g*64+bj*16+j."""
    perm = np.empty(H, np.int64)
    for g in range(G):
        for j in range(16):
            for bj in range(4):
                perm[g * 64 + j * 4 + bj] = g * 64 + bj * 16 + j
    return perm


def _wbt_perm(Wb):
    """WBT[p, load, slot, n] = Wb.T[k, n] for chunk ci=load*16+slot,
    k = g*4096 + i*64 + j, i = bi*8 + p//16, j = bj*16 + p%16,
    ci = g*32 + bi*4 + bj."""
    wbt = Wb.T  # [K, NL]
    p = np.arange(128)
    ip, jp = p // 16, p % 16
    rows = np.empty((384, 128), np.int64)
    for g in range(G):
        for bi in range(8):
            for bj in range(4):
                ci = g * 32 + bi * 4 + bj
                rows[ci] = g * 4096 + (bi * 8 + ip) * 64 + (bj * 16 + jp)
    w = wbt[rows]                                # [384, 128, NL]
    return np.ascontiguousarray(
        w.reshape(12, 32, 128, NL).transpose(2, 0, 1, 3)).astype(BF)


_CACHE = {}


def _prep_core_inputs(c, sequence_output, attention, mention_mask, Wh, bh, Wt, bt,
                      Wb, bb, mention_idx, hts):
    b, half = c // 2, c % 2
    seq_b = np.ascontiguousarray(sequence_output[b])              # [L, H]
    idx = mention_idx[b].astype(np.int64).reshape(EM)             # [96]
    mask = mention_mask[b].astype(np.float32)                     # [E, M]
    denom = mask.sum(-1)                                          # [E]

    emg = np.ascontiguousarray(seq_b[idx])                        # [96, H]
    amg = np.ascontiguousarray(
        attention[b][:, idx, :].transpose(1, 0, 2).reshape(EM, HL))

    sume = np.zeros((EM, E), np.float32)
    for e in range(E):
        for m in range(M):
            sume[e * M + m, e] = mask[e, m]

    hts_c = hts[b, half * R:(half + 1) * R].astype(np.int64)      # [R, 2]
    s = 1.0 / np.sqrt(np.float32(NH))
    wm = (mask / denom[:, None] * s).reshape(EM)                  # [96]
    whh = wm[:, None] * (hts_c[None, :, 0] == (np.arange(EM) // M)[:, None])
    wtt = wm[:, None] * (hts_c[None, :, 1] == (np.arange(EM) // M)[:, None])
    ohh = (hts_c[None, :, 0] == np.arange(E)[:, None]).astype(np.float32)
    oht = (hts_c[None, :, 1] == np.arange(E)[:, None]).astype(np.float32)

    tp = _CACHE.setdefault("tperm", _t_perm())
    w1h = np.ascontiguousarray(Wh[:, :H].T)                       # [768, 768]
    w2h = np.ascontiguousarray(Wh[:, H:].T)
    w1t = np.ascontiguousarray(Wt[tp, :H].T)
    w2t = np.ascontiguousarray(Wt[tp, H:].T)

    if "wbt" not in _CACHE:
        _CACHE["wbt"] = _wbt_perm(Wb)

    return {
        "EMG": emg.astype(np.float32),
        "SUME": sume.astype(BF),
        "AMG": amg.astype(BF),
        "WHH": whh.astype(BF), "WTT": wtt.astype(BF),
        "OHH": ohh.astype(BF), "OHT": oht.astype(BF),
        "SEQ": seq_b.astype(BF),
        "W1H": w1h.astype(BF), "W2H": w2h.astype(BF),
        "W1T": w1t.astype(BF), "W2T": w2t.astype(BF),
        "WBT": _CACHE["wbt"],
        "BHS": np.ascontiguousarray(bh.reshape(6, 128).T).astype(np.float32),
        "BTS": np.ascontiguousarray(bt[tp].reshape(6, 128).T).astype(np.float32),
        "BBS": bb.reshape(NL, 1).astype(np.float32),
        "XD": np.zeros((8, G, 8, R), BF),
        "ZD": np.zeros((16, G, 4, R), BF),
    }


def kernel(sequence_output, attention, mention_mask, Wh, bh, Wt, bt, Wb, bb,
           mention_idx, hts):
    if "nc" not in _CACHE:
        _CACHE["nc"] = _build_program()
    nc = _CACHE["nc"]

    args = (np.asarray(sequence_output, np.float32), np.asarray(attention, np.float32),
            np.asarray(mention_mask, np.float32), np.asarray(Wh, np.float32),
            np.asarray(bh, np.float32), np.asarray(Wt, np.float32),
            np.asarray(bt, np.float32), np.asarray(Wb, np.float32),
            np.asarray(bb, np.float32), np.asarray(mention_idx),
            np.asarray(hts))
    _CACHE.pop("wbt", None)   # Wb may differ between calls
    in_maps = [_prep_core_inputs(c, *args) for c in range(8)]
    res = None
    for attempt in range(4):
        try:
            res = run_bass_kernel_spmd(nc, in_maps, list(range(8))).results
            break
        except Exception:
            # transient NRT_EXEC_UNIT_UNRECOVERABLE is occasionally raised by
            # the runtime; back off briefly and retry
            if attempt == 3:
                raise
            import time
            time.sleep(2.0 * (attempt + 1))

    out = np.empty((B, P, NL), np.float32)
    for c in range(8):
        b, half = c // 2, c % 2
        out[b, half * R:(half + 1) * R, :] = np.asarray(res[c]["OUT"]).T
    return out
